# revision 7
# baseline (speedup 1.0000x reference)
"""ClusterGCN (3-layer GCN, sum-aggregation) on 8 Trainium2 NeuronCores.

Strategy (hardcoded for B=2, N=50000, F=H=128, E=800000, 8 cores):
  - core c: destination shard c (6250 nodes), BOTH batches. Tables are
    batch-interleaved [N, 256]: row n = [h(b0,n,:) | h(b1,n,:)] in bf16, so
    one dma_gather index fetches both batches' source rows (512B). The SWDGE
    gather cost is per-index (measured flat in element width up to 1KB), so
    interleaving halves system-wide descriptor work vs per-batch sharding.
  - Reassociate each layer: A @ (h @ W) == (A @ h) @ W: aggregate first
    (segment-sum over edges), then one dense 128x128 matmul per batch.
  - Edges sorted by (dst_tile, src_bucket, dst, src) into 128-slot chunks;
    each chunk -> one is_equal one-hot [slot, dst_rel] and two accumulating
    matmuls (one per batch) into PSUM agg tiles [feat, dst]. Sources split
    into lo/hi buckets (int16 gather indices, offset table views).
  - Gather calls are <=1024 idx (Q7 scratch limit), cycled over SWDGE
    queues 0..3 (different Q7 CPU pairs; ~25% faster than one queue).
  - SPMD: one instruction stream for all 8 cores -> the call schedule is
    canonical (per-(tile,bucket) max chunk count over shards); each shard
    pads its own chunks with idx 0 / dst_rel 255 (one-hot all-zero).
  - BatchNorm is training-mode over all B*N rows: per-core bn_stats/bn_aggr,
    then an 8-core AllReduce of (mean, E[x^2]).
  - After BN+ReLU tiles are transposed back row-major (bf16) and AllGathered
    (all 8 cores) into the next layer's gather table.
"""

import math

import numpy as np

P = 128
FEAT = 128
CPC = 8  # max chunks per gather call (8*128 = 1024 idx, Q7 scratch limit)


class Cfg:
    def __init__(self, n_nodes=50000, batch=2, eps=1e-5):
        self.N = n_nodes
        self.SHARD = n_nodes // 8  # 6250
        self.BATCH = batch
        self.HALF = n_nodes // 2
        assert self.HALF <= 32767
        self.TILES = math.ceil(self.SHARD / P)  # 49
        self.VALID_LAST = self.SHARD - (self.TILES - 1) * P  # 106
        self.EPS = eps
        self.LAYERS = 3
        self.USE_AR = True
        self.USE_AG = True
        self.BF16 = True


def _wrap16(stream):
    """[n] idx stream -> [128, n/16] wrapped col-major, replicated x8."""
    return np.tile(stream.reshape(-1, 16).T, (8, 1))


def build_schedule(cfg, edge_index):
    """Canonical dst-sorted chunk schedule shared by all 8 shards.

    Returns (calls, tile_chunks, wi_list, wd_list):
      calls: list of (tile, bucket, chunk0, nch, icol) gather calls
      tile_chunks: [TILES] chunks per tile
      wi_list[q]: [128, n_chunks*8] i16 wrapped gather idx for shard q
      wd_list[q]: [128, n_chunks] f32 per-chunk dst_rel (along partitions)
    """
    row = np.asarray(edge_index[0]).astype(np.int64)
    col = np.asarray(edge_index[1]).astype(np.int64)

    # per shard: dict[(tile, bucket)] -> (idx16 [n], drel [n])
    groups = []
    for q in range(8):
        base = q * cfg.SHARD
        m = (col >= base) & (col < base + cfg.SHARD)
        r = row[m]
        c = col[m] - base
        t = c // P
        drel = c % P
        bkt = (r >= cfg.HALF).astype(np.int64)
        order = np.lexsort((r, drel, bkt, t))
        r, t, drel, bkt = r[order], t[order], drel[order], bkt[order]
        idx16 = np.where(bkt == 1, r - cfg.HALF, r).astype(np.int16)
        g = {}
        key = t * 2 + bkt
        bounds = np.flatnonzero(np.append(True, key[1:] != key[:-1]))
        bounds = np.append(bounds, len(key))
        for j in range(len(bounds) - 1):
            s, e = int(bounds[j]), int(bounds[j + 1])
            g[(int(t[s]), int(bkt[s]))] = (idx16[s:e], drel[s:e].astype(np.float32))
        groups.append(g)

    # canonical chunk counts: per (tile, bucket) max over shards
    kmax = {}
    for t in range(cfg.TILES):
        for b in (0, 1):
            n = max(len(g.get((t, b), ((), ()))[0]) for g in groups)
            kmax[(t, b)] = max(1, math.ceil(n / P))

    calls = []
    tile_chunks = np.zeros(cfg.TILES, np.int64)
    chunk_of = {}  # (tile,bucket) -> first chunk index
    c0 = 0
    icol = 0
    for t in range(cfg.TILES):
        for b in (0, 1):
            k = kmax[(t, b)]
            chunk_of[(t, b)] = c0
            tile_chunks[t] += k
            for s2 in range(0, k, CPC):
                nch = min(CPC, k - s2)
                calls.append((t, b, c0 + s2, nch, icol))
                icol += nch * 8
            c0 += k
    nch_total = c0

    wi_list, wd_list = [], []
    for q in range(8):
        wi = np.zeros((128, nch_total * 8), np.int16)
        wd = np.full((128, nch_total), 255.0, np.float32)
        for (t, b), cc0 in chunk_of.items():
            idx16, drel = groups[q].get((t, b), (np.zeros(0, np.int16),
                                                 np.zeros(0, np.float32)))
            k = kmax[(t, b)]
            pi = np.zeros(k * P, np.int16)
            pd = np.full(k * P, 255.0, np.float32)
            pi[:len(idx16)] = idx16
            pd[:len(drel)] = drel
            wi[:, cc0 * 8:(cc0 + k) * 8] = _wrap16(pi)
            wd[:, cc0:cc0 + k] = pd.reshape(k, P).T
        wi_list.append(wi)
        wd_list.append(wd)
    return calls, tile_chunks, wi_list, wd_list


# ---------------------------------------------------------------- bass kernel


def build_nc(cfg, calls, tile_chunks, shapes):
    import concourse.bacc as bacc
    import concourse.bass as bass
    import concourse.tile as tile
    from concourse import mybir

    f32 = mybir.dt.float32
    bf16 = mybir.dt.bfloat16
    i16 = mybir.dt.int16
    tdt = bf16 if cfg.BF16 else f32
    TW = 2 * FEAT  # interleaved table width (256)

    nc = bacc.Bacc("TRN2", target_bir_lowering=False, debug=False,
                   num_devices=8, num_swdge_queues=4)

    x_tab = nc.dram_tensor("x_tab", [cfg.N, TW], tdt, kind="ExternalInput")
    wi_d = nc.dram_tensor("wi", list(shapes["wi"]), i16, kind="ExternalInput")
    wd_d = nc.dram_tensor("wd", list(shapes["wd"]), f32, kind="ExternalInput")
    w_dr = [nc.dram_tensor(f"W{i+1}", [FEAT, FEAT if i < 2 else 1], f32,
                           kind="ExternalInput") for i in range(3)]
    b_dr = [nc.dram_tensor(f"b{i+1}", [FEAT if i < 2 else 1], f32,
                           kind="ExternalInput") for i in range(3)]
    gb_dr = [(nc.dram_tensor(f"gamma{i+1}", [FEAT], f32, kind="ExternalInput"),
              nc.dram_tensor(f"beta{i+1}", [FEAT], f32, kind="ExternalInput"))
             for i in range(2)]
    iota_p_d = nc.dram_tensor("iota_p", [P, P], f32, kind="ExternalInput")
    ident_d = nc.dram_tensor("ident", [P, P], f32, kind="ExternalInput")
    out_d = nc.dram_tensor("out", [cfg.BATCH, cfg.SHARD], f32, kind="ExternalOutput")

    htab = [nc.dram_tensor(f"htab{i}", [cfg.N, TW], tdt, kind="Internal")
            for i in range(2)]
    shard_out = [nc.dram_tensor(f"shard_out{i}", [cfg.SHARD, TW], tdt,
                                kind="Internal") for i in range(2)]
    stat_in = [nc.dram_tensor(f"stat_in{i}", [P, 2], f32, kind="Internal")
               for i in range(2)]
    stat_out = [nc.dram_tensor(f"stat_out{i}", [P, 2], f32, kind="Internal")
                for i in range(2)]

    AluOp = mybir.AluOpType
    ActF = mybir.ActivationFunctionType

    def bcast_inner(ap, inner):
        return bass.AP(tensor=ap.tensor, offset=ap.offset,
                       ap=[list(ap.ap[0]), list(ap.ap[1]), [0, inner]])

    def bcast_rep(ap, reps):
        return bass.AP(tensor=ap.tensor, offset=ap.offset,
                       ap=[list(ap.ap[0]), [0, reps], list(ap.ap[1])])

    with tile.TileContext(nc) as tc:
        with (
            tc.tile_pool(name="consts", bufs=1) as consts,
            tc.tile_pool(name="gw", bufs=4) as gwp,
            tc.tile_pool(name="ohp", bufs=4) as ohp,
            tc.tile_pool(name="aggp", bufs=2) as aggp,
            tc.tile_pool(name="hraw", bufs=1) as hrawp,
            tc.tile_pool(name="statp", bufs=2) as statp,
            tc.tile_pool(name="small", bufs=8) as small,
            tc.tile_pool(name="p2", bufs=3) as p2p,
            tc.tile_pool(name="outp", bufs=1) as outp,
            tc.tile_pool(name="ps_agg", bufs=2, space="PSUM") as ps_agg,
            tc.tile_pool(name="ps_h", bufs=2, space="PSUM") as ps_h,
            tc.tile_pool(name="ps_t", bufs=2, space="PSUM") as ps_t,
        ):
            wi_sb = consts.tile(list(shapes["wi"]), i16, tag="wi")
            nc.sync.dma_start(out=wi_sb[:], in_=wi_d[:])
            wd_sb = consts.tile(list(shapes["wd"]), f32, tag="wd")
            nc.sync.dma_start(out=wd_sb[:], in_=wd_d[:])
            w_sb = []
            for i, wdr in enumerate(w_dr):
                t = consts.tile([P, FEAT if i < 2 else 1], f32, tag=f"w{i}")
                nc.sync.dma_start(out=t[:], in_=wdr[:])
                w_sb.append(t)
            b_sb = []
            for i, bd in enumerate(b_dr):
                t = consts.tile([P, 1], f32, tag=f"b{i}")
                if i < 2:
                    nc.sync.dma_start(out=t[:], in_=bd[:, None])
                else:
                    nc.sync.dma_start(out=t[:], in_=bd[:].to_broadcast([P, 1]))
                b_sb.append(t)
            gb_sb = []
            for i, (gd, bd) in enumerate(gb_dr):
                tg = consts.tile([P, 1], f32, tag=f"g{i}")
                nc.sync.dma_start(out=tg[:], in_=gd[:, None])
                tb = consts.tile([P, 1], f32, tag=f"be{i}")
                nc.sync.dma_start(out=tb[:], in_=bd[:, None])
                gb_sb.append((tg, tb))
            iota_p = consts.tile([P, P], f32, tag="iota_p")
            nc.sync.dma_start(out=iota_p[:], in_=iota_p_d[:])
            ident = consts.tile([P, P], f32, tag="ident")
            nc.sync.dma_start(out=ident[:], in_=ident_d[:])
            eps_sb = consts.tile([P, 1], f32, tag="eps")
            nc.vector.memset(eps_sb[:], cfg.EPS)

            for layer in range(cfg.LAYERS):
                table = x_tab if layer == 0 else htab[layer - 1]
                is_last = layer == cfg.LAYERS - 1
                if not is_last:
                    hraw = [hrawp.tile([P, cfg.TILES * P], f32,
                                       tag=f"hraw{b}", name=f"hraw{b}")
                            for b in range(2)]
                    stat_t = statp.tile([P, 2 * cfg.TILES, 6], f32, tag="stats")
                else:
                    out_sb = [outp.tile([P, cfg.TILES], f32,
                                        tag=f"outsb{b}", name=f"outsb{b}")
                              for b in range(2)]

                cur_tile = -1
                done_chunks = 0
                agg_ps = None
                qn = 0
                for (tt, bb, c0, nch, icol) in calls:
                    if tt != cur_tile:
                        cur_tile = tt
                        done_chunks = 0
                        agg_ps = [ps_agg.tile([P, P], f32, tag=f"agg{b}",
                                              name=f"agg{b}")
                                  for b in range(2)]
                    gt = gwp.tile([P, CPC, TW], tdt, tag="gw")
                    src = table[:, :] if bb == 0 else table[cfg.HALF:, :]
                    nc.gpsimd.dma_gather(
                        gt[:, :nch, :], src,
                        wi_sb[:, icol:icol + nch * 8],
                        nch * P, nch * P, TW,
                        queue_num=qn,
                    )
                    qn = (qn + 1) % 4
                    oh = ohp.tile([P, CPC * P], tdt, tag="oh")
                    nc.vector.tensor_tensor(
                        out=oh[:, :nch * P],
                        in0=bcast_inner(wd_sb[:, c0:c0 + nch], P),
                        in1=bcast_rep(iota_p[:], nch),
                        op=AluOp.is_equal,
                    )
                    total = int(tile_chunks[tt])
                    for j in range(nch):
                        first = done_chunks == 0
                        last = done_chunks == total - 1
                        for b in range(2):
                            nc.tensor.matmul(
                                agg_ps[b][:, :],
                                lhsT=gt[:, j, b * FEAT:(b + 1) * FEAT],
                                rhs=oh[:, j * P:(j + 1) * P],
                                start=first, stop=last,
                            )
                        done_chunks += 1

                    if done_chunks == total:
                        valid = cfg.VALID_LAST if tt == cfg.TILES - 1 else P
                        for b in range(2):
                            agg_sb = aggp.tile([P, P], f32, tag=f"aggsb{b}")
                            nc.vector.tensor_copy(out=agg_sb[:], in_=agg_ps[b][:])
                            if not is_last:
                                h_ps = ps_h.tile([P, P], f32, tag="hps")
                                nc.tensor.matmul(
                                    h_ps[:], lhsT=w_sb[layer][:], rhs=agg_sb[:],
                                    start=True, stop=True,
                                )
                                nc.vector.tensor_scalar_add(
                                    out=hraw[b][:, tt * P:tt * P + P],
                                    in0=h_ps[:], scalar1=b_sb[layer][:],
                                )
                                nc.vector.bn_stats(
                                    out=stat_t[:, 2 * tt + b, :],
                                    in_=hraw[b][:, tt * P:tt * P + valid],
                                )
                            else:
                                o_ps = ps_h.tile([P, P], f32, tag="hps")
                                o_ps = o_ps[:, 0:1]
                                nc.tensor.matmul(
                                    o_ps[:], lhsT=agg_sb[:], rhs=w_sb[2][:],
                                    start=True, stop=True,
                                )
                                nc.vector.tensor_scalar_add(
                                    out=out_sb[b][:, tt:tt + 1], in0=o_ps[:],
                                    scalar1=b_sb[2][:],
                                )

                if not is_last:
                    mv = small.tile([P, 2], f32, tag="mv")
                    nc.vector.bn_aggr(out=mv[:], in_=stat_t[:, :, :])
                    sloc = small.tile([P, 2], f32, tag="sloc")
                    nc.vector.tensor_copy(out=sloc[:, 0:1], in_=mv[:, 0:1])
                    nc.vector.tensor_tensor(
                        out=sloc[:, 1:2], in0=mv[:, 0:1], in1=mv[:, 0:1],
                        op=AluOp.mult,
                    )
                    nc.vector.tensor_add(
                        out=sloc[:, 1:2], in0=sloc[:, 1:2], in1=mv[:, 1:2]
                    )
                    nc.sync.dma_start(out=stat_in[layer][:], in_=sloc[:])
                    if cfg.USE_AR:
                        nc.gpsimd.collective_compute(
                            "AllReduce", AluOp.add,
                            replica_groups=[[0, 1, 2, 3, 4, 5, 6, 7]],
                            ins=[stat_in[layer][:]], outs=[stat_out[layer][:]],
                        )
                    else:
                        nc.sync.dma_start(out=stat_out[layer][:],
                                          in_=stat_in[layer][:])
                    sglob = small.tile([P, 2], f32, tag="sglob")
                    nc.sync.dma_start(out=sglob[:], in_=stat_out[layer][:])
                    nc.scalar.mul(out=sglob[:], in_=sglob[:],
                                  mul=0.125 if cfg.USE_AR else 1.0)
                    var = small.tile([P, 1], f32, tag="var")
                    nc.vector.tensor_tensor(
                        out=var[:], in0=sglob[:, 0:1], in1=sglob[:, 0:1],
                        op=AluOp.mult,
                    )
                    nc.vector.tensor_sub(out=var[:], in0=sglob[:, 1:2], in1=var[:])
                    rstd = small.tile([P, 1], f32, tag="rstd")
                    nc.scalar.activation(out=rstd[:], in_=var[:], func=ActF.Sqrt,
                                         bias=eps_sb[:])
                    nc.vector.reciprocal(out=rstd[:], in_=rstd[:])
                    scal = small.tile([P, 1], f32, tag="scal")
                    nc.vector.tensor_tensor(
                        out=scal[:], in0=gb_sb[layer][0][:], in1=rstd[:],
                        op=AluOp.mult,
                    )
                    shif = small.tile([P, 1], f32, tag="shif")
                    nc.vector.tensor_tensor(
                        out=shif[:], in0=sglob[:, 0:1], in1=scal[:], op=AluOp.mult,
                    )
                    nc.vector.tensor_sub(out=shif[:], in0=gb_sb[layer][1][:],
                                         in1=shif[:])
                    for t in range(cfg.TILES):
                        valid = cfg.VALID_LAST if t == cfg.TILES - 1 else P
                        for b in range(2):
                            hbn = p2p.tile([P, P], f32, tag="hbn")
                            nc.scalar.activation(
                                out=hbn[:], in_=hraw[b][:, t * P:(t + 1) * P],
                                func=ActF.Relu, bias=shif[:], scale=scal[:],
                            )
                            t_ps = ps_t.tile([P, P], f32, tag="tps")
                            nc.tensor.transpose(out=t_ps[:], in_=hbn[:],
                                                identity=ident[:])
                            hrow = p2p.tile([P, P], tdt, tag="hrow")
                            nc.vector.tensor_copy(out=hrow[:], in_=t_ps[:])
                            nc.sync.dma_start(
                                out=shard_out[layer][t * P:t * P + valid,
                                                     b * FEAT:(b + 1) * FEAT],
                                in_=hrow[:valid, :],
                            )
                    if cfg.USE_AG:
                        nc.gpsimd.collective_compute(
                            "AllGather", AluOp.bypass,
                            replica_groups=[[0, 1, 2, 3, 4, 5, 6, 7]],
                            ins=[shard_out[layer][:]], outs=[htab[layer][:]],
                        )
                    else:
                        nc.sync.dma_start(out=htab[layer][0:cfg.SHARD, :],
                                          in_=shard_out[layer][:])
                else:
                    nfull = cfg.TILES - 1
                    for b in range(2):
                        nc.sync.dma_start(
                            out=out_d[b, 0:nfull * P].rearrange(
                                "(t p) -> p t", p=P),
                            in_=out_sb[b][:, 0:nfull],
                        )
                        nc.sync.dma_start(
                            out=out_d[b, nfull * P:cfg.SHARD, None],
                            in_=out_sb[b][:cfg.VALID_LAST, nfull:nfull + 1],
                        )

    nc.compile()
    return nc


# ---------------------------------------------------------------- host + run


def run_gcn(cfg, inputs, trace=False):
    import ml_dtypes
    from concourse.bass_utils import run_bass_kernel_spmd

    tnp = ml_dtypes.bfloat16 if cfg.BF16 else np.float32
    x = np.asarray(inputs["x"], dtype=np.float32)
    edge_index = np.asarray(inputs["edge_index"])
    calls, tile_chunks, wi_list, wd_list = build_schedule(cfg, edge_index)
    shapes = {"wi": wi_list[0].shape, "wd": wd_list[0].shape}
    nc = build_nc(cfg, calls, tile_chunks, shapes)

    # interleaved x table: row n = [x(b0,n,:) | x(b1,n,:)]
    x_il = np.concatenate([x[0], x[1]], axis=1).astype(tnp)
    iota_p = np.tile(np.arange(P, dtype=np.float32), (P, 1))
    ident = np.eye(P, dtype=np.float32)
    common = {
        "W1": np.asarray(inputs["W1"], np.float32),
        "W2": np.asarray(inputs["W2"], np.float32),
        "W3": np.asarray(inputs["W3"], np.float32),
        "b1": np.asarray(inputs["b1"], np.float32),
        "b2": np.asarray(inputs["b2"], np.float32),
        "b3": np.asarray(inputs["b3"], np.float32),
        "gamma1": np.asarray(inputs["gamma1"], np.float32),
        "beta1": np.asarray(inputs["beta1"], np.float32),
        "gamma2": np.asarray(inputs["gamma2"], np.float32),
        "beta2": np.asarray(inputs["beta2"], np.float32),
        "iota_p": iota_p,
        "ident": ident,
        "x_tab": x_il,
    }
    in_maps = []
    for c in range(8):
        m = dict(common)
        m["wi"] = wi_list[c]
        m["wd"] = wd_list[c]
        in_maps.append(m)

    try:
        res = run_bass_kernel_spmd(nc, in_maps, core_ids=list(range(8)), trace=trace)
    except ModuleNotFoundError:
        res = run_bass_kernel_spmd(nc, in_maps, core_ids=list(range(8)), trace=False)
    out = np.empty((cfg.BATCH, cfg.N), np.float32)
    for c in range(8):
        out[:, c * cfg.SHARD:(c + 1) * cfg.SHARD] = res.results[c]["out"]
    return out, res


def kernel(**inputs) -> np.ndarray:
    cfg = Cfg()
    out, _ = run_gcn(cfg, inputs, trace=False)
    return out


# revision 10
# speedup vs baseline: 1.3900x; 1.3900x over previous
"""ClusterGCN (3-layer GCN, sum-aggregation) on 8 Trainium2 NeuronCores.

Strategy (hardcoded for B=2, N=50000, F=H=128, E=800000, 8 cores):
  - core c: destination shard c (6250 nodes), BOTH batches. Tables are
    batch-interleaved [N, 256]: row n = [h(b0,n,:) | h(b1,n,:)] in bf16, so
    one dma_gather index fetches both batches' source rows (512B). The SWDGE
    gather cost is per-index (measured flat in element width up to 1KB), so
    interleaving halves system-wide descriptor work vs per-batch sharding.
  - Reassociate each layer: A @ (h @ W) == (A @ h) @ W: aggregate first
    (segment-sum over edges), then one dense 128x128 matmul per batch.
  - Edges sorted by (dst_tile, src_bucket, dst, src) into 128-slot chunks;
    each chunk -> one is_equal one-hot [slot, dst_rel] and two accumulating
    matmuls (one per batch) into PSUM agg tiles [feat, dst]. Sources split
    into lo/hi buckets (int16 gather indices, offset table views).
  - Gather calls are <=1024 idx (Q7 scratch limit), cycled over SWDGE
    queues 0..3 (different Q7 CPU pairs; ~25% faster than one queue).
  - SPMD: one instruction stream for all 8 cores -> the call schedule is
    canonical (per-(tile,bucket) max chunk count over shards); each shard
    pads its own chunks with idx 0 / dst_rel 255 (one-hot all-zero).
  - BatchNorm is training-mode over all B*N rows: per-core bn_stats/bn_aggr,
    then an 8-core AllReduce of (mean, E[x^2]).
  - After BN+ReLU tiles are transposed back row-major (bf16) and AllGathered
    (all 8 cores) into the next layer's gather table.
"""

import math

import numpy as np

P = 128
FEAT = 128
CPC = 8  # max chunks per gather call (8*128 = 1024 idx, Q7 scratch limit)


class Cfg:
    def __init__(self, n_nodes=50000, batch=2, eps=1e-5):
        self.N = n_nodes
        self.SHARD = n_nodes // 8  # 6250
        self.BATCH = batch
        self.HALF = n_nodes // 2
        assert self.HALF <= 32767
        self.TILES = math.ceil(self.SHARD / P)  # 49
        self.VALID_LAST = self.SHARD - (self.TILES - 1) * P  # 106
        self.EPS = eps
        self.LAYERS = 3
        self.USE_AR = True
        self.USE_AG = True
        self.BF16 = True


def _wrap16(stream):
    """[n] idx stream -> [128, n/16] wrapped col-major, replicated x8."""
    return np.tile(stream.reshape(-1, 16).T, (8, 1))


def build_schedule(cfg, edge_index, remap=None):
    """Canonical dst-sorted chunk schedule shared by all 8 shards.

    remap: optional vectorized fn mapping global source ids to table rows
    (used for the half-block htab layout of layers 2-3).

    Returns (calls, tile_chunks, wi_list, wd_list):
      calls: list of (tile, bucket, chunk0, nch, icol) gather calls
      tile_chunks: [TILES] chunks per tile
      wi_list[q]: [128, n_chunks*8] i16 wrapped gather idx for shard q
      wd_list[q]: [128, n_chunks] f32 per-chunk dst_rel (along partitions)
    """
    row = np.asarray(edge_index[0]).astype(np.int64)
    col = np.asarray(edge_index[1]).astype(np.int64)
    if remap is not None:
        row = remap(row)

    # per shard: dict[(tile, bucket)] -> (idx16 [n], drel [n])
    groups = []
    for q in range(8):
        base = q * cfg.SHARD
        m = (col >= base) & (col < base + cfg.SHARD)
        r = row[m]
        c = col[m] - base
        t = c // P
        drel = c % P
        bkt = (r >= cfg.HALF).astype(np.int64)
        order = np.lexsort((r, drel, bkt, t))
        r, t, drel, bkt = r[order], t[order], drel[order], bkt[order]
        idx16 = np.where(bkt == 1, r - cfg.HALF, r).astype(np.int16)
        g = {}
        key = t * 2 + bkt
        bounds = np.flatnonzero(np.append(True, key[1:] != key[:-1]))
        bounds = np.append(bounds, len(key))
        for j in range(len(bounds) - 1):
            s, e = int(bounds[j]), int(bounds[j + 1])
            g[(int(t[s]), int(bkt[s]))] = (idx16[s:e], drel[s:e].astype(np.float32))
        groups.append(g)

    # canonical chunk counts: per (tile, bucket) max over shards
    kmax = {}
    for t in range(cfg.TILES):
        for b in (0, 1):
            n = max(len(g.get((t, b), ((), ()))[0]) for g in groups)
            kmax[(t, b)] = max(1, math.ceil(n / P))

    calls = []
    tile_chunks = np.zeros(cfg.TILES, np.int64)
    chunk_of = {}  # (tile,bucket) -> first chunk index
    c0 = 0
    icol = 0
    for t in range(cfg.TILES):
        for b in (0, 1):
            k = kmax[(t, b)]
            chunk_of[(t, b)] = c0
            tile_chunks[t] += k
            for s2 in range(0, k, CPC):
                nch = min(CPC, k - s2)
                calls.append((t, b, c0 + s2, nch, icol))
                icol += nch * 8
            c0 += k
    nch_total = c0

    wi_list, wd_list = [], []
    for q in range(8):
        wi = np.zeros((128, nch_total * 8), np.int16)
        wd = np.full((128, nch_total), 255.0, np.float32)
        for (t, b), cc0 in chunk_of.items():
            idx16, drel = groups[q].get((t, b), (np.zeros(0, np.int16),
                                                 np.zeros(0, np.float32)))
            k = kmax[(t, b)]
            pi = np.zeros(k * P, np.int16)
            pd = np.full(k * P, 255.0, np.float32)
            pi[:len(idx16)] = idx16
            pd[:len(drel)] = drel
            wi[:, cc0 * 8:(cc0 + k) * 8] = _wrap16(pi)
            wd[:, cc0:cc0 + k] = pd.reshape(k, P).T
        wi_list.append(wi)
        wd_list.append(wd)
    return calls, tile_chunks, wi_list, wd_list


# ---------------------------------------------------------------- bass kernel


def build_nc(cfg, scheds, shapes):
    import concourse.bacc as bacc
    import concourse.bass as bass
    import concourse.tile as tile
    from concourse import mybir

    f32 = mybir.dt.float32
    bf16 = mybir.dt.bfloat16
    i16 = mybir.dt.int16
    tdt = bf16 if cfg.BF16 else f32
    TW = 2 * FEAT  # interleaved table width (256)

    nc = bacc.Bacc("TRN2", target_bir_lowering=False, debug=False,
                   num_devices=8, num_swdge_queues=4)

    x_tab = nc.dram_tensor("x_tab", [cfg.N, TW], tdt, kind="ExternalInput")
    wix_d = nc.dram_tensor("wix", list(shapes["wix"]), i16, kind="ExternalInput")
    wih_d = nc.dram_tensor("wih", list(shapes["wih"]), i16, kind="ExternalInput")
    wdx_d = nc.dram_tensor("wdx", list(shapes["wdx"]), f32, kind="ExternalInput")
    wdh_d = nc.dram_tensor("wdh", list(shapes["wdh"]), f32, kind="ExternalInput")
    w_dr = [nc.dram_tensor(f"W{i+1}", [FEAT, FEAT if i < 2 else 1], f32,
                           kind="ExternalInput") for i in range(3)]
    b_dr = [nc.dram_tensor(f"b{i+1}", [FEAT if i < 2 else 1], f32,
                           kind="ExternalInput") for i in range(3)]
    gb_dr = [(nc.dram_tensor(f"gamma{i+1}", [FEAT], f32, kind="ExternalInput"),
              nc.dram_tensor(f"beta{i+1}", [FEAT], f32, kind="ExternalInput"))
             for i in range(2)]
    iota_p_d = nc.dram_tensor("iota_p", [P, P], f32, kind="ExternalInput")
    ident_d = nc.dram_tensor("ident", [P, P], f32, kind="ExternalInput")
    out_d = nc.dram_tensor("out", [cfg.BATCH, cfg.SHARD], f32, kind="ExternalOutput")

    htab = [nc.dram_tensor(f"htab{i}", [cfg.N, TW], tdt, kind="Internal")
            for i in range(2)]
    shard_out = [nc.dram_tensor(f"shard_out{i}", [cfg.SHARD, TW], tdt,
                                kind="Internal") for i in range(2)]
    stat_in = [nc.dram_tensor(f"stat_in{i}", [P, 2], f32, kind="Internal")
               for i in range(2)]
    stat_out = [nc.dram_tensor(f"stat_out{i}", [P, 2], f32, kind="Internal")
                for i in range(2)]

    AluOp = mybir.AluOpType
    ActF = mybir.ActivationFunctionType

    def bcast_inner(ap, inner):
        return bass.AP(tensor=ap.tensor, offset=ap.offset,
                       ap=[list(ap.ap[0]), list(ap.ap[1]), [0, inner]])

    def bcast_rep(ap, reps):
        return bass.AP(tensor=ap.tensor, offset=ap.offset,
                       ap=[list(ap.ap[0]), [0, reps], list(ap.ap[1])])

    with tile.TileContext(nc) as tc:
        with (
            tc.tile_pool(name="consts", bufs=1) as consts,
            tc.tile_pool(name="gw", bufs=6) as gwp,
            tc.tile_pool(name="ohp", bufs=6) as ohp,
            tc.tile_pool(name="aggp", bufs=2) as aggp,
            tc.tile_pool(name="hraw", bufs=1) as hrawp,
            tc.tile_pool(name="statp", bufs=2) as statp,
            tc.tile_pool(name="small", bufs=8) as small,
            tc.tile_pool(name="p2", bufs=3) as p2p,
            tc.tile_pool(name="outp", bufs=1) as outp,
            tc.tile_pool(name="ps_agg", bufs=2, space="PSUM") as ps_agg,
            tc.tile_pool(name="ps_h", bufs=2, space="PSUM") as ps_h,
            tc.tile_pool(name="ps_t", bufs=2, space="PSUM") as ps_t,
        ):
            wix_sb = consts.tile(list(shapes["wix"]), i16, tag="wix")
            nc.sync.dma_start(out=wix_sb[:], in_=wix_d[:])
            wih_sb = consts.tile(list(shapes["wih"]), i16, tag="wih")
            nc.sync.dma_start(out=wih_sb[:], in_=wih_d[:])
            wdx_sb = consts.tile(list(shapes["wdx"]), f32, tag="wdx")
            nc.sync.dma_start(out=wdx_sb[:], in_=wdx_d[:])
            wdh_sb = consts.tile(list(shapes["wdh"]), f32, tag="wdh")
            nc.sync.dma_start(out=wdh_sb[:], in_=wdh_d[:])
            w_sb = []
            for i, wdr in enumerate(w_dr):
                t = consts.tile([P, FEAT if i < 2 else 1], f32, tag=f"w{i}")
                nc.sync.dma_start(out=t[:], in_=wdr[:])
                w_sb.append(t)
            b_sb = []
            for i, bd in enumerate(b_dr):
                t = consts.tile([P, 1], f32, tag=f"b{i}")
                if i < 2:
                    nc.sync.dma_start(out=t[:], in_=bd[:, None])
                else:
                    nc.sync.dma_start(out=t[:], in_=bd[:].to_broadcast([P, 1]))
                b_sb.append(t)
            gb_sb = []
            for i, (gd, bd) in enumerate(gb_dr):
                tg = consts.tile([P, 1], f32, tag=f"g{i}")
                nc.sync.dma_start(out=tg[:], in_=gd[:, None])
                tb = consts.tile([P, 1], f32, tag=f"be{i}")
                nc.sync.dma_start(out=tb[:], in_=bd[:, None])
                gb_sb.append((tg, tb))
            iota_p = consts.tile([P, P], f32, tag="iota_p")
            nc.sync.dma_start(out=iota_p[:], in_=iota_p_d[:])
            ident = consts.tile([P, P], f32, tag="ident")
            nc.sync.dma_start(out=ident[:], in_=ident_d[:])
            eps_sb = consts.tile([P, 1], f32, tag="eps")
            nc.vector.memset(eps_sb[:], cfg.EPS)

            for layer in range(cfg.LAYERS):
                table = x_tab if layer == 0 else htab[layer - 1]
                calls, tile_chunks, wi_sb, wd_sb = (
                    (scheds[0][0], scheds[0][1], wix_sb, wdx_sb) if layer == 0
                    else (scheds[1][0], scheds[1][1], wih_sb, wdh_sb))
                is_last = layer == cfg.LAYERS - 1
                if not is_last:
                    hraw = [hrawp.tile([P, cfg.TILES * P], f32,
                                       tag=f"hraw{b}", name=f"hraw{b}")
                            for b in range(2)]
                    stat_t = statp.tile([P, 2 * cfg.TILES, 6], f32, tag="stats")
                else:
                    out_sb = [outp.tile([P, cfg.TILES], f32,
                                        tag=f"outsb{b}", name=f"outsb{b}")
                              for b in range(2)]

                cur_tile = -1
                done_chunks = 0
                agg_ps = None
                qn = 0
                for (tt, bb, c0, nch, icol) in calls:
                    if tt != cur_tile:
                        cur_tile = tt
                        done_chunks = 0
                        agg_ps = [ps_agg.tile([P, P], f32, tag=f"agg{b}",
                                              name=f"agg{b}")
                                  for b in range(2)]
                    gt = gwp.tile([P, CPC, TW], tdt, tag="gw")
                    src = table[:, :] if bb == 0 else table[cfg.HALF:, :]
                    nc.gpsimd.dma_gather(
                        gt[:, :nch, :], src,
                        wi_sb[:, icol:icol + nch * 8],
                        nch * P, nch * P, TW,
                        queue_num=qn,
                    )
                    qn = (qn + 1) % 4
                    oh = ohp.tile([P, CPC * P], tdt, tag="oh")
                    nc.vector.tensor_tensor(
                        out=oh[:, :nch * P],
                        in0=bcast_inner(wd_sb[:, c0:c0 + nch], P),
                        in1=bcast_rep(iota_p[:], nch),
                        op=AluOp.is_equal,
                    )
                    total = int(tile_chunks[tt])
                    for j in range(nch):
                        first = done_chunks == 0
                        last = done_chunks == total - 1
                        for b in range(2):
                            nc.tensor.matmul(
                                agg_ps[b][:, :],
                                lhsT=gt[:, j, b * FEAT:(b + 1) * FEAT],
                                rhs=oh[:, j * P:(j + 1) * P],
                                start=first, stop=last,
                            )
                        done_chunks += 1

                    if done_chunks == total:
                        valid = cfg.VALID_LAST if tt == cfg.TILES - 1 else P
                        for b in range(2):
                            agg_sb = aggp.tile([P, P], f32, tag=f"aggsb{b}")
                            nc.vector.tensor_copy(out=agg_sb[:], in_=agg_ps[b][:])
                            if not is_last:
                                h_ps = ps_h.tile([P, P], f32, tag="hps")
                                nc.tensor.matmul(
                                    h_ps[:], lhsT=w_sb[layer][:], rhs=agg_sb[:],
                                    start=True, stop=True,
                                )
                                nc.vector.tensor_scalar_add(
                                    out=hraw[b][:, tt * P:tt * P + P],
                                    in0=h_ps[:], scalar1=b_sb[layer][:],
                                )
                                nc.vector.bn_stats(
                                    out=stat_t[:, 2 * tt + b, :],
                                    in_=hraw[b][:, tt * P:tt * P + valid],
                                )
                            else:
                                o_ps = ps_h.tile([P, P], f32, tag="hps")
                                o_ps = o_ps[:, 0:1]
                                nc.tensor.matmul(
                                    o_ps[:], lhsT=agg_sb[:], rhs=w_sb[2][:],
                                    start=True, stop=True,
                                )
                                nc.vector.tensor_scalar_add(
                                    out=out_sb[b][:, tt:tt + 1], in0=o_ps[:],
                                    scalar1=b_sb[2][:],
                                )

                if not is_last:
                    mv = small.tile([P, 2], f32, tag="mv")
                    nc.vector.bn_aggr(out=mv[:], in_=stat_t[:, :, :])
                    sloc = small.tile([P, 2], f32, tag="sloc")
                    nc.vector.tensor_copy(out=sloc[:, 0:1], in_=mv[:, 0:1])
                    nc.vector.tensor_tensor(
                        out=sloc[:, 1:2], in0=mv[:, 0:1], in1=mv[:, 0:1],
                        op=AluOp.mult,
                    )
                    nc.vector.tensor_add(
                        out=sloc[:, 1:2], in0=sloc[:, 1:2], in1=mv[:, 1:2]
                    )
                    nc.sync.dma_start(out=stat_in[layer][:], in_=sloc[:])
                    if cfg.USE_AR:
                        nc.gpsimd.collective_compute(
                            "AllReduce", AluOp.add,
                            replica_groups=[[0, 1, 2, 3, 4, 5, 6, 7]],
                            ins=[stat_in[layer][:]], outs=[stat_out[layer][:]],
                        )
                    else:
                        nc.sync.dma_start(out=stat_out[layer][:],
                                          in_=stat_in[layer][:])
                    sglob = small.tile([P, 2], f32, tag="sglob")
                    nc.sync.dma_start(out=sglob[:], in_=stat_out[layer][:])
                    nc.scalar.mul(out=sglob[:], in_=sglob[:],
                                  mul=0.125 if cfg.USE_AR else 1.0)
                    var = small.tile([P, 1], f32, tag="var")
                    nc.vector.tensor_tensor(
                        out=var[:], in0=sglob[:, 0:1], in1=sglob[:, 0:1],
                        op=AluOp.mult,
                    )
                    nc.vector.tensor_sub(out=var[:], in0=sglob[:, 1:2], in1=var[:])
                    rstd = small.tile([P, 1], f32, tag="rstd")
                    nc.scalar.activation(out=rstd[:], in_=var[:], func=ActF.Sqrt,
                                         bias=eps_sb[:])
                    nc.vector.reciprocal(out=rstd[:], in_=rstd[:])
                    scal = small.tile([P, 1], f32, tag="scal")
                    nc.vector.tensor_tensor(
                        out=scal[:], in0=gb_sb[layer][0][:], in1=rstd[:],
                        op=AluOp.mult,
                    )
                    shif = small.tile([P, 1], f32, tag="shif")
                    nc.vector.tensor_tensor(
                        out=shif[:], in0=sglob[:, 0:1], in1=scal[:], op=AluOp.mult,
                    )
                    nc.vector.tensor_sub(out=shif[:], in0=gb_sb[layer][1][:],
                                         in1=shif[:])
                    HT = 25  # first-half tiles (rows 0:3200)
                    R1 = HT * P
                    R2 = cfg.SHARD - R1
                    for half, (t0, t1) in enumerate(((0, HT), (HT, cfg.TILES))):
                        for t in range(t0, t1):
                            valid = cfg.VALID_LAST if t == cfg.TILES - 1 else P
                            for b in range(2):
                                hbn = p2p.tile([P, P], f32, tag="hbn")
                                nc.scalar.activation(
                                    out=hbn[:], in_=hraw[b][:, t * P:(t + 1) * P],
                                    func=ActF.Relu, bias=shif[:], scale=scal[:],
                                )
                                t_ps = ps_t.tile([P, P], f32, tag="tps")
                                nc.tensor.transpose(out=t_ps[:], in_=hbn[:],
                                                    identity=ident[:])
                                hrow = p2p.tile([P, P], tdt, tag="hrow")
                                nc.vector.tensor_copy(out=hrow[:], in_=t_ps[:])
                                nc.sync.dma_start(
                                    out=shard_out[layer][t * P:t * P + valid,
                                                         b * FEAT:(b + 1) * FEAT],
                                    in_=hrow[:valid, :],
                                )
                        if cfg.USE_AG:
                            ht = htab[layer]
                            if half == 0:
                                in_ap = shard_out[layer][0:R1, :]
                                out_ap = ht[0:8 * R1, :]
                            else:
                                in_ap = shard_out[layer][R1:cfg.SHARD, :]
                                out_ap = ht[8 * R1:cfg.N, :]
                            nc.gpsimd.collective_compute(
                                "AllGather", AluOp.bypass,
                                replica_groups=[[0, 1, 2, 3, 4, 5, 6, 7]],
                                ins=[in_ap], outs=[out_ap],
                            )
                    if not cfg.USE_AG:
                        nc.sync.dma_start(out=htab[layer][0:cfg.SHARD, :],
                                          in_=shard_out[layer][:])
                else:
                    nfull = cfg.TILES - 1
                    for b in range(2):
                        nc.sync.dma_start(
                            out=out_d[b, 0:nfull * P].rearrange(
                                "(t p) -> p t", p=P),
                            in_=out_sb[b][:, 0:nfull],
                        )
                        nc.sync.dma_start(
                            out=out_d[b, nfull * P:cfg.SHARD, None],
                            in_=out_sb[b][:cfg.VALID_LAST, nfull:nfull + 1],
                        )

    nc.compile()
    return nc


# ---------------------------------------------------------------- host + run


def run_gcn(cfg, inputs, trace=False):
    import ml_dtypes
    from concourse.bass_utils import run_bass_kernel_spmd

    tnp = ml_dtypes.bfloat16 if cfg.BF16 else np.float32
    x = np.asarray(inputs["x"], dtype=np.float32)
    edge_index = np.asarray(inputs["edge_index"])
    R1 = 25 * P  # 3200: first-half rows per shard in the htab block layout

    def remap_h(r):
        q, rr = r // cfg.SHARD, r % cfg.SHARD
        return np.where(rr < R1, q * R1 + rr,
                        8 * R1 + q * (cfg.SHARD - R1) + (rr - R1))

    calls_x, tcx, wix_list, wdx_list = build_schedule(cfg, edge_index)
    calls_h, tch, wih_list, wdh_list = build_schedule(cfg, edge_index, remap_h)
    # dst-side chunk layout must match between the two schedules for wd to be
    # shared; bucket membership differs, so use the h-schedule's wd and ALSO
    # the x-schedule's own wd: keep both by merging into one wd of max width.
    shapes = {"wix": wix_list[0].shape, "wih": wih_list[0].shape,
              "wdx": wdx_list[0].shape, "wdh": wdh_list[0].shape}
    nc = build_nc(cfg, (
        (calls_x, tcx), (calls_h, tch)), shapes)

    # interleaved x table: row n = [x(b0,n,:) | x(b1,n,:)]
    x_il = np.concatenate([x[0], x[1]], axis=1).astype(tnp)
    iota_p = np.tile(np.arange(P, dtype=np.float32), (P, 1))
    ident = np.eye(P, dtype=np.float32)
    common = {
        "W1": np.asarray(inputs["W1"], np.float32),
        "W2": np.asarray(inputs["W2"], np.float32),
        "W3": np.asarray(inputs["W3"], np.float32),
        "b1": np.asarray(inputs["b1"], np.float32),
        "b2": np.asarray(inputs["b2"], np.float32),
        "b3": np.asarray(inputs["b3"], np.float32),
        "gamma1": np.asarray(inputs["gamma1"], np.float32),
        "beta1": np.asarray(inputs["beta1"], np.float32),
        "gamma2": np.asarray(inputs["gamma2"], np.float32),
        "beta2": np.asarray(inputs["beta2"], np.float32),
        "iota_p": iota_p,
        "ident": ident,
        "x_tab": x_il,
    }
    in_maps = []
    for c in range(8):
        m = dict(common)
        m["wix"] = wix_list[c]
        m["wih"] = wih_list[c]
        m["wdx"] = wdx_list[c]
        m["wdh"] = wdh_list[c]
        in_maps.append(m)

    try:
        res = run_bass_kernel_spmd(nc, in_maps, core_ids=list(range(8)), trace=trace)
    except ModuleNotFoundError:
        res = run_bass_kernel_spmd(nc, in_maps, core_ids=list(range(8)), trace=False)
    out = np.empty((cfg.BATCH, cfg.N), np.float32)
    for c in range(8):
        out[:, c * cfg.SHARD:(c + 1) * cfg.SHARD] = res.results[c]["out"]
    return out, res


def kernel(**inputs) -> np.ndarray:
    cfg = Cfg()
    out, _ = run_gcn(cfg, inputs, trace=False)
    return out


# revision 11
# speedup vs baseline: 1.4069x; 1.0122x over previous
"""ClusterGCN (3-layer GCN, sum-aggregation) on 8 Trainium2 NeuronCores.

Strategy (hardcoded for B=2, N=50000, F=H=128, E=800000, 8 cores):
  - core c: destination shard c (6250 nodes), BOTH batches. Tables are
    batch-interleaved [N, 256]: row n = [h(b0,n,:) | h(b1,n,:)] in bf16, so
    one dma_gather index fetches both batches' source rows (512B). The SWDGE
    gather cost is per-index (measured flat in element width up to 1KB), so
    interleaving halves system-wide descriptor work vs per-batch sharding.
  - Reassociate each layer: A @ (h @ W) == (A @ h) @ W: aggregate first
    (segment-sum over edges), then one dense 128x128 matmul per batch.
  - Edges sorted by (dst_tile, src_bucket, dst, src) into 128-slot chunks;
    each chunk -> one is_equal one-hot [slot, dst_rel] and two accumulating
    matmuls (one per batch) into PSUM agg tiles [feat, dst]. Sources split
    into lo/hi buckets (int16 gather indices, offset table views).
  - Gather calls are <=1024 idx (Q7 scratch limit), cycled over SWDGE
    queues 0..3 (different Q7 CPU pairs; ~25% faster than one queue).
  - SPMD: one instruction stream for all 8 cores -> the call schedule is
    canonical (per-(tile,bucket) max chunk count over shards); each shard
    pads its own chunks with idx 0 / dst_rel 255 (one-hot all-zero).
  - BatchNorm is training-mode over all B*N rows: per-core bn_stats/bn_aggr,
    then an 8-core AllReduce of (mean, E[x^2]).
  - After BN+ReLU tiles are transposed back row-major (bf16) and AllGathered
    (all 8 cores) into the next layer's gather table.
"""

import math

import numpy as np

P = 128
FEAT = 128
CPC = 8  # max chunks per gather call (8*128 = 1024 idx, Q7 scratch limit)


class Cfg:
    def __init__(self, n_nodes=50000, batch=2, eps=1e-5):
        self.N = n_nodes
        self.SHARD = n_nodes // 8  # 6250
        self.BATCH = batch
        self.HALF = n_nodes // 2
        assert self.HALF <= 32767
        self.TILES = math.ceil(self.SHARD / P)  # 49
        self.VALID_LAST = self.SHARD - (self.TILES - 1) * P  # 106
        self.EPS = eps
        self.LAYERS = 3
        self.USE_AR = True
        self.USE_AG = True
        self.BF16 = True


def _wrap16(stream):
    """[n] idx stream -> [128, n/16] wrapped col-major, replicated x8."""
    return np.tile(stream.reshape(-1, 16).T, (8, 1))


def build_schedule(cfg, edge_index, remap=None, half=None):
    """Canonical dst-sorted chunk schedule shared by all 8 shards.

    remap: optional vectorized fn mapping global source ids to table rows
    (used for the half-block htab layout of layers 2-3).

    Returns (calls, tile_chunks, wi_list, wd_list):
      calls: list of (tile, bucket, chunk0, nch, icol) gather calls
      tile_chunks: [TILES] chunks per tile
      wi_list[q]: [128, n_chunks*8] i16 wrapped gather idx for shard q
      wd_list[q]: [128, n_chunks] f32 per-chunk dst_rel (along partitions)
    """
    row = np.asarray(edge_index[0]).astype(np.int64)
    col = np.asarray(edge_index[1]).astype(np.int64)
    if remap is not None:
        row = remap(row)
    if half is None:
        half = cfg.HALF

    # per shard: dict[(tile, bucket)] -> (idx16 [n], drel [n])
    groups = []
    for q in range(8):
        base = q * cfg.SHARD
        m = (col >= base) & (col < base + cfg.SHARD)
        r = row[m]
        c = col[m] - base
        t = c // P
        drel = c % P
        bkt = (r >= half).astype(np.int64)
        order = np.lexsort((r, drel, bkt, t))
        r, t, drel, bkt = r[order], t[order], drel[order], bkt[order]
        idx16 = np.where(bkt == 1, r - half, r).astype(np.int16)
        g = {}
        key = t * 2 + bkt
        bounds = np.flatnonzero(np.append(True, key[1:] != key[:-1]))
        bounds = np.append(bounds, len(key))
        for j in range(len(bounds) - 1):
            s, e = int(bounds[j]), int(bounds[j + 1])
            g[(int(t[s]), int(bkt[s]))] = (idx16[s:e], drel[s:e].astype(np.float32))
        groups.append(g)

    # canonical chunk counts: per (tile, bucket) max over shards
    kmax = {}
    for t in range(cfg.TILES):
        for b in (0, 1):
            n = max(len(g.get((t, b), ((), ()))[0]) for g in groups)
            kmax[(t, b)] = max(1, math.ceil(n / P))

    calls = []
    tile_chunks = np.zeros(cfg.TILES, np.int64)
    chunk_of = {}  # (tile,bucket) -> first chunk index
    c0 = 0
    icol = 0
    for t in range(cfg.TILES):
        for b in (0, 1):
            k = kmax[(t, b)]
            chunk_of[(t, b)] = c0
            tile_chunks[t] += k
            for s2 in range(0, k, CPC):
                nch = min(CPC, k - s2)
                calls.append((t, b, c0 + s2, nch, icol))
                icol += nch * 8
            c0 += k
    nch_total = c0

    wi_list, wd_list = [], []
    for q in range(8):
        wi = np.zeros((128, nch_total * 8), np.int16)
        wd = np.full((128, nch_total), 255.0, np.float32)
        for (t, b), cc0 in chunk_of.items():
            idx16, drel = groups[q].get((t, b), (np.zeros(0, np.int16),
                                                 np.zeros(0, np.float32)))
            k = kmax[(t, b)]
            pi = np.zeros(k * P, np.int16)
            pd = np.full(k * P, 255.0, np.float32)
            pi[:len(idx16)] = idx16
            pd[:len(drel)] = drel
            wi[:, cc0 * 8:(cc0 + k) * 8] = _wrap16(pi)
            wd[:, cc0:cc0 + k] = pd.reshape(k, P).T
        wi_list.append(wi)
        wd_list.append(wd)
    return calls, tile_chunks, wi_list, wd_list


# ---------------------------------------------------------------- bass kernel


def build_nc(cfg, scheds, shapes):
    import concourse.bacc as bacc
    import concourse.bass as bass
    import concourse.tile as tile
    from concourse import mybir

    f32 = mybir.dt.float32
    bf16 = mybir.dt.bfloat16
    i16 = mybir.dt.int16
    tdt = bf16 if cfg.BF16 else f32
    TW = 2 * FEAT  # interleaved table width (256)

    nc = bacc.Bacc("TRN2", target_bir_lowering=False, debug=False,
                   num_devices=8, num_swdge_queues=4)

    x_tab = nc.dram_tensor("x_tab", [cfg.N, TW], tdt, kind="ExternalInput")
    wix_d = nc.dram_tensor("wix", list(shapes["wix"]), i16, kind="ExternalInput")
    wih_d = nc.dram_tensor("wih", list(shapes["wih"]), i16, kind="ExternalInput")
    wdx_d = nc.dram_tensor("wdx", list(shapes["wdx"]), f32, kind="ExternalInput")
    wdh_d = nc.dram_tensor("wdh", list(shapes["wdh"]), f32, kind="ExternalInput")
    w_dr = [nc.dram_tensor(f"W{i+1}", [FEAT, FEAT if i < 2 else 1], f32,
                           kind="ExternalInput") for i in range(3)]
    b_dr = [nc.dram_tensor(f"b{i+1}", [FEAT if i < 2 else 1], f32,
                           kind="ExternalInput") for i in range(3)]
    gb_dr = [(nc.dram_tensor(f"gamma{i+1}", [FEAT], f32, kind="ExternalInput"),
              nc.dram_tensor(f"beta{i+1}", [FEAT], f32, kind="ExternalInput"))
             for i in range(2)]
    iota_p_d = nc.dram_tensor("iota_p", [P, P], f32, kind="ExternalInput")
    ident_d = nc.dram_tensor("ident", [P, P], f32, kind="ExternalInput")
    out_d = nc.dram_tensor("out", [cfg.BATCH, cfg.SHARD], f32, kind="ExternalOutput")

    htab = [nc.dram_tensor(f"htab{i}", [cfg.N, TW], tdt, kind="Internal")
            for i in range(2)]
    shard_out = [nc.dram_tensor(f"shard_out{i}", [cfg.SHARD, TW], tdt,
                                kind="Internal") for i in range(2)]
    stat_in = [nc.dram_tensor(f"stat_in{i}", [P, 2], f32, kind="Internal")
               for i in range(2)]
    stat_out = [nc.dram_tensor(f"stat_out{i}", [P, 2], f32, kind="Internal")
                for i in range(2)]

    AluOp = mybir.AluOpType
    ActF = mybir.ActivationFunctionType

    def bcast_inner(ap, inner):
        return bass.AP(tensor=ap.tensor, offset=ap.offset,
                       ap=[list(ap.ap[0]), list(ap.ap[1]), [0, inner]])

    def bcast_rep(ap, reps):
        return bass.AP(tensor=ap.tensor, offset=ap.offset,
                       ap=[list(ap.ap[0]), [0, reps], list(ap.ap[1])])

    with tile.TileContext(nc) as tc:
        with (
            tc.tile_pool(name="consts", bufs=1) as consts,
            tc.tile_pool(name="gw", bufs=6) as gwp,
            tc.tile_pool(name="ohp", bufs=6) as ohp,
            tc.tile_pool(name="aggp", bufs=2) as aggp,
            tc.tile_pool(name="hraw", bufs=1) as hrawp,
            tc.tile_pool(name="statp", bufs=2) as statp,
            tc.tile_pool(name="small", bufs=8) as small,
            tc.tile_pool(name="p2", bufs=3) as p2p,
            tc.tile_pool(name="outp", bufs=1) as outp,
            tc.tile_pool(name="ps_agg", bufs=2, space="PSUM") as ps_agg,
            tc.tile_pool(name="ps_h", bufs=2, space="PSUM") as ps_h,
            tc.tile_pool(name="ps_t", bufs=2, space="PSUM") as ps_t,
        ):
            wix_sb = consts.tile(list(shapes["wix"]), i16, tag="wix")
            nc.sync.dma_start(out=wix_sb[:], in_=wix_d[:])
            wih_sb = consts.tile(list(shapes["wih"]), i16, tag="wih")
            nc.sync.dma_start(out=wih_sb[:], in_=wih_d[:])
            wdx_sb = consts.tile(list(shapes["wdx"]), f32, tag="wdx")
            nc.sync.dma_start(out=wdx_sb[:], in_=wdx_d[:])
            wdh_sb = consts.tile(list(shapes["wdh"]), f32, tag="wdh")
            nc.sync.dma_start(out=wdh_sb[:], in_=wdh_d[:])
            w_sb = []
            for i, wdr in enumerate(w_dr):
                t = consts.tile([P, FEAT if i < 2 else 1], f32, tag=f"w{i}")
                nc.sync.dma_start(out=t[:], in_=wdr[:])
                w_sb.append(t)
            b_sb = []
            for i, bd in enumerate(b_dr):
                t = consts.tile([P, 1], f32, tag=f"b{i}")
                if i < 2:
                    nc.sync.dma_start(out=t[:], in_=bd[:, None])
                else:
                    nc.sync.dma_start(out=t[:], in_=bd[:].to_broadcast([P, 1]))
                b_sb.append(t)
            gb_sb = []
            for i, (gd, bd) in enumerate(gb_dr):
                tg = consts.tile([P, 1], f32, tag=f"g{i}")
                nc.sync.dma_start(out=tg[:], in_=gd[:, None])
                tb = consts.tile([P, 1], f32, tag=f"be{i}")
                nc.sync.dma_start(out=tb[:], in_=bd[:, None])
                gb_sb.append((tg, tb))
            iota_p = consts.tile([P, P], f32, tag="iota_p")
            nc.sync.dma_start(out=iota_p[:], in_=iota_p_d[:])
            ident = consts.tile([P, P], f32, tag="ident")
            nc.sync.dma_start(out=ident[:], in_=ident_d[:])
            eps_sb = consts.tile([P, 1], f32, tag="eps")
            nc.vector.memset(eps_sb[:], cfg.EPS)

            for layer in range(cfg.LAYERS):
                table = x_tab if layer == 0 else htab[layer - 1]
                calls, tile_chunks, wi_sb, wd_sb = (
                    (scheds[0][0], scheds[0][1], wix_sb, wdx_sb) if layer == 0
                    else (scheds[1][0], scheds[1][1], wih_sb, wdh_sb))
                half = cfg.HALF if layer == 0 else 8 * 25 * P
                is_last = layer == cfg.LAYERS - 1
                if not is_last:
                    hraw = [hrawp.tile([P, cfg.TILES * P], f32,
                                       tag=f"hraw{b}", name=f"hraw{b}")
                            for b in range(2)]
                    stat_t = statp.tile([P, 2 * cfg.TILES, 6], f32, tag="stats")
                else:
                    out_sb = [outp.tile([P, cfg.TILES], f32,
                                        tag=f"outsb{b}", name=f"outsb{b}")
                              for b in range(2)]

                cur_tile = -1
                done_chunks = 0
                agg_ps = None
                qn = 0
                for (tt, bb, c0, nch, icol) in calls:
                    if tt != cur_tile:
                        cur_tile = tt
                        done_chunks = 0
                        agg_ps = [ps_agg.tile([P, P], f32, tag=f"agg{b}",
                                              name=f"agg{b}")
                                  for b in range(2)]
                    gt = gwp.tile([P, CPC, TW], tdt, tag="gw")
                    src = table[0:half, :] if bb == 0 else table[half:, :]
                    nc.gpsimd.dma_gather(
                        gt[:, :nch, :], src,
                        wi_sb[:, icol:icol + nch * 8],
                        nch * P, nch * P, TW,
                        queue_num=qn,
                    )
                    qn = (qn + 1) % 4
                    oh = ohp.tile([P, CPC * P], tdt, tag="oh")
                    nc.vector.tensor_tensor(
                        out=oh[:, :nch * P],
                        in0=bcast_inner(wd_sb[:, c0:c0 + nch], P),
                        in1=bcast_rep(iota_p[:], nch),
                        op=AluOp.is_equal,
                    )
                    total = int(tile_chunks[tt])
                    for j in range(nch):
                        first = done_chunks == 0
                        last = done_chunks == total - 1
                        for b in range(2):
                            nc.tensor.matmul(
                                agg_ps[b][:, :],
                                lhsT=gt[:, j, b * FEAT:(b + 1) * FEAT],
                                rhs=oh[:, j * P:(j + 1) * P],
                                start=first, stop=last,
                            )
                        done_chunks += 1

                    if done_chunks == total:
                        valid = cfg.VALID_LAST if tt == cfg.TILES - 1 else P
                        for b in range(2):
                            agg_sb = aggp.tile([P, P], f32, tag=f"aggsb{b}")
                            nc.vector.tensor_copy(out=agg_sb[:], in_=agg_ps[b][:])
                            if not is_last:
                                h_ps = ps_h.tile([P, P], f32, tag="hps")
                                nc.tensor.matmul(
                                    h_ps[:], lhsT=w_sb[layer][:], rhs=agg_sb[:],
                                    start=True, stop=True,
                                )
                                nc.vector.tensor_scalar_add(
                                    out=hraw[b][:, tt * P:tt * P + P],
                                    in0=h_ps[:], scalar1=b_sb[layer][:],
                                )
                                nc.vector.bn_stats(
                                    out=stat_t[:, 2 * tt + b, :],
                                    in_=hraw[b][:, tt * P:tt * P + valid],
                                )
                            else:
                                o_ps = ps_h.tile([P, P], f32, tag="hps")
                                o_ps = o_ps[:, 0:1]
                                nc.tensor.matmul(
                                    o_ps[:], lhsT=agg_sb[:], rhs=w_sb[2][:],
                                    start=True, stop=True,
                                )
                                nc.vector.tensor_scalar_add(
                                    out=out_sb[b][:, tt:tt + 1], in0=o_ps[:],
                                    scalar1=b_sb[2][:],
                                )

                if not is_last:
                    mv = small.tile([P, 2], f32, tag="mv")
                    nc.vector.bn_aggr(out=mv[:], in_=stat_t[:, :, :])
                    sloc = small.tile([P, 2], f32, tag="sloc")
                    nc.vector.tensor_copy(out=sloc[:, 0:1], in_=mv[:, 0:1])
                    nc.vector.tensor_tensor(
                        out=sloc[:, 1:2], in0=mv[:, 0:1], in1=mv[:, 0:1],
                        op=AluOp.mult,
                    )
                    nc.vector.tensor_add(
                        out=sloc[:, 1:2], in0=sloc[:, 1:2], in1=mv[:, 1:2]
                    )
                    nc.sync.dma_start(out=stat_in[layer][:], in_=sloc[:])
                    if cfg.USE_AR:
                        nc.gpsimd.collective_compute(
                            "AllReduce", AluOp.add,
                            replica_groups=[[0, 1, 2, 3, 4, 5, 6, 7]],
                            ins=[stat_in[layer][:]], outs=[stat_out[layer][:]],
                        )
                    else:
                        nc.sync.dma_start(out=stat_out[layer][:],
                                          in_=stat_in[layer][:])
                    sglob = small.tile([P, 2], f32, tag="sglob")
                    nc.sync.dma_start(out=sglob[:], in_=stat_out[layer][:])
                    nc.scalar.mul(out=sglob[:], in_=sglob[:],
                                  mul=0.125 if cfg.USE_AR else 1.0)
                    var = small.tile([P, 1], f32, tag="var")
                    nc.vector.tensor_tensor(
                        out=var[:], in0=sglob[:, 0:1], in1=sglob[:, 0:1],
                        op=AluOp.mult,
                    )
                    nc.vector.tensor_sub(out=var[:], in0=sglob[:, 1:2], in1=var[:])
                    rstd = small.tile([P, 1], f32, tag="rstd")
                    nc.scalar.activation(out=rstd[:], in_=var[:], func=ActF.Sqrt,
                                         bias=eps_sb[:])
                    nc.vector.reciprocal(out=rstd[:], in_=rstd[:])
                    scal = small.tile([P, 1], f32, tag="scal")
                    nc.vector.tensor_tensor(
                        out=scal[:], in0=gb_sb[layer][0][:], in1=rstd[:],
                        op=AluOp.mult,
                    )
                    shif = small.tile([P, 1], f32, tag="shif")
                    nc.vector.tensor_tensor(
                        out=shif[:], in0=sglob[:, 0:1], in1=scal[:], op=AluOp.mult,
                    )
                    nc.vector.tensor_sub(out=shif[:], in0=gb_sb[layer][1][:],
                                         in1=shif[:])
                    HT = 25  # first-half tiles (rows 0:3200)
                    R1 = HT * P
                    R2 = cfg.SHARD - R1
                    for half, (t0, t1) in enumerate(((0, HT), (HT, cfg.TILES))):
                        for t in range(t0, t1):
                            valid = cfg.VALID_LAST if t == cfg.TILES - 1 else P
                            for b in range(2):
                                hbn = p2p.tile([P, P], f32, tag="hbn")
                                nc.scalar.activation(
                                    out=hbn[:], in_=hraw[b][:, t * P:(t + 1) * P],
                                    func=ActF.Relu, bias=shif[:], scale=scal[:],
                                )
                                t_ps = ps_t.tile([P, P], f32, tag="tps")
                                nc.tensor.transpose(out=t_ps[:], in_=hbn[:],
                                                    identity=ident[:])
                                hrow = p2p.tile([P, P], tdt, tag="hrow")
                                nc.vector.tensor_copy(out=hrow[:], in_=t_ps[:])
                                nc.sync.dma_start(
                                    out=shard_out[layer][t * P:t * P + valid,
                                                         b * FEAT:(b + 1) * FEAT],
                                    in_=hrow[:valid, :],
                                )
                        if cfg.USE_AG:
                            ht = htab[layer]
                            if half == 0:
                                in_ap = shard_out[layer][0:R1, :]
                                out_ap = ht[0:8 * R1, :]
                            else:
                                in_ap = shard_out[layer][R1:cfg.SHARD, :]
                                out_ap = ht[8 * R1:cfg.N, :]
                            nc.gpsimd.collective_compute(
                                "AllGather", AluOp.bypass,
                                replica_groups=[[0, 1, 2, 3, 4, 5, 6, 7]],
                                ins=[in_ap], outs=[out_ap],
                            )
                    if not cfg.USE_AG:
                        nc.sync.dma_start(out=htab[layer][0:cfg.SHARD, :],
                                          in_=shard_out[layer][:])
                else:
                    nfull = cfg.TILES - 1
                    for b in range(2):
                        nc.sync.dma_start(
                            out=out_d[b, 0:nfull * P].rearrange(
                                "(t p) -> p t", p=P),
                            in_=out_sb[b][:, 0:nfull],
                        )
                        nc.sync.dma_start(
                            out=out_d[b, nfull * P:cfg.SHARD, None],
                            in_=out_sb[b][:cfg.VALID_LAST, nfull:nfull + 1],
                        )

    nc.compile()
    return nc


# ---------------------------------------------------------------- host + run


def run_gcn(cfg, inputs, trace=False):
    import ml_dtypes
    from concourse.bass_utils import run_bass_kernel_spmd

    tnp = ml_dtypes.bfloat16 if cfg.BF16 else np.float32
    x = np.asarray(inputs["x"], dtype=np.float32)
    edge_index = np.asarray(inputs["edge_index"])
    R1 = 25 * P  # 3200: first-half rows per shard in the htab block layout

    def remap_h(r):
        q, rr = r // cfg.SHARD, r % cfg.SHARD
        return np.where(rr < R1, q * R1 + rr,
                        8 * R1 + q * (cfg.SHARD - R1) + (rr - R1))

    calls_x, tcx, wix_list, wdx_list = build_schedule(cfg, edge_index)
    calls_h, tch, wih_list, wdh_list = build_schedule(cfg, edge_index, remap_h,
                                                      half=8 * R1)
    # dst-side chunk layout must match between the two schedules for wd to be
    # shared; bucket membership differs, so use the h-schedule's wd and ALSO
    # the x-schedule's own wd: keep both by merging into one wd of max width.
    shapes = {"wix": wix_list[0].shape, "wih": wih_list[0].shape,
              "wdx": wdx_list[0].shape, "wdh": wdh_list[0].shape}
    nc = build_nc(cfg, (
        (calls_x, tcx), (calls_h, tch)), shapes)

    # interleaved x table: row n = [x(b0,n,:) | x(b1,n,:)]
    x_il = np.concatenate([x[0], x[1]], axis=1).astype(tnp)
    iota_p = np.tile(np.arange(P, dtype=np.float32), (P, 1))
    ident = np.eye(P, dtype=np.float32)
    common = {
        "W1": np.asarray(inputs["W1"], np.float32),
        "W2": np.asarray(inputs["W2"], np.float32),
        "W3": np.asarray(inputs["W3"], np.float32),
        "b1": np.asarray(inputs["b1"], np.float32),
        "b2": np.asarray(inputs["b2"], np.float32),
        "b3": np.asarray(inputs["b3"], np.float32),
        "gamma1": np.asarray(inputs["gamma1"], np.float32),
        "beta1": np.asarray(inputs["beta1"], np.float32),
        "gamma2": np.asarray(inputs["gamma2"], np.float32),
        "beta2": np.asarray(inputs["beta2"], np.float32),
        "iota_p": iota_p,
        "ident": ident,
        "x_tab": x_il,
    }
    in_maps = []
    for c in range(8):
        m = dict(common)
        m["wix"] = wix_list[c]
        m["wih"] = wih_list[c]
        m["wdx"] = wdx_list[c]
        m["wdh"] = wdh_list[c]
        in_maps.append(m)

    try:
        res = run_bass_kernel_spmd(nc, in_maps, core_ids=list(range(8)), trace=trace)
    except ModuleNotFoundError:
        res = run_bass_kernel_spmd(nc, in_maps, core_ids=list(range(8)), trace=False)
    out = np.empty((cfg.BATCH, cfg.N), np.float32)
    for c in range(8):
        out[:, c * cfg.SHARD:(c + 1) * cfg.SHARD] = res.results[c]["out"]
    return out, res


def kernel(**inputs) -> np.ndarray:
    cfg = Cfg()
    out, _ = run_gcn(cfg, inputs, trace=False)
    return out


# revision 12
# speedup vs baseline: 1.4379x; 1.0220x over previous
"""ClusterGCN (3-layer GCN, sum-aggregation) on 8 Trainium2 NeuronCores.

Strategy (hardcoded for B=2, N=50000, F=H=128, E=800000, 8 cores):
  - core c: destination shard c (6250 nodes), BOTH batches. Tables are
    batch-interleaved [N, 256]: row n = [h(b0,n,:) | h(b1,n,:)] in bf16, so
    one dma_gather index fetches both batches' source rows (512B). The SWDGE
    gather cost is per-index (measured flat in element width up to 1KB), so
    interleaving halves system-wide descriptor work vs per-batch sharding.
  - Reassociate each layer: A @ (h @ W) == (A @ h) @ W: aggregate first
    (segment-sum over edges), then one dense 128x128 matmul per batch.
  - Edges sorted by (dst_tile, src_bucket, dst, src) into 128-slot chunks;
    each chunk -> one is_equal one-hot [slot, dst_rel] and two accumulating
    matmuls (one per batch) into PSUM agg tiles [feat, dst]. Sources split
    into lo/hi buckets (int16 gather indices, offset table views).
  - Gather calls are <=1024 idx (Q7 scratch limit), cycled over SWDGE
    queues 0..3 (different Q7 CPU pairs; ~25% faster than one queue).
  - SPMD: one instruction stream for all 8 cores -> the call schedule is
    canonical (per-(tile,bucket) max chunk count over shards); each shard
    pads its own chunks with idx 0 / dst_rel 255 (one-hot all-zero).
  - BatchNorm is training-mode over all B*N rows: per-core bn_stats/bn_aggr,
    then an 8-core AllReduce of (mean, E[x^2]).
  - After BN+ReLU tiles are transposed back row-major (bf16) and AllGathered
    (all 8 cores) into the next layer's gather table.
"""

import math

import numpy as np

P = 128
FEAT = 128
CPC = 8  # max chunks per gather call (8*128 = 1024 idx, Q7 scratch limit)


class Cfg:
    def __init__(self, n_nodes=50000, batch=2, eps=1e-5):
        self.N = n_nodes
        self.SHARD = n_nodes // 8  # 6250
        self.BATCH = batch
        self.HALF = n_nodes // 2
        assert self.HALF <= 32767
        self.TILES = math.ceil(self.SHARD / P)  # 49
        self.VALID_LAST = self.SHARD - (self.TILES - 1) * P  # 106
        self.EPS = eps
        self.LAYERS = 3
        self.USE_AR = True
        self.USE_AG = True
        self.BF16 = True


def _wrap16(stream):
    """[n] idx stream -> [128, n/16] wrapped col-major, replicated x8."""
    return np.tile(stream.reshape(-1, 16).T, (8, 1))


def build_schedule(cfg, edge_index, remap=None, half=None):
    """Canonical dst-sorted chunk schedule shared by all 8 shards.

    remap: optional vectorized fn mapping global source ids to table rows
    (used for the half-block htab layout of layers 2-3).

    Returns (calls, tile_chunks, wi_list, wd_list):
      calls: list of (tile, bucket, chunk0, nch, icol) gather calls
      tile_chunks: [TILES] chunks per tile
      wi_list[q]: [128, n_chunks*8] i16 wrapped gather idx for shard q
      wd_list[q]: [128, n_chunks] f32 per-chunk dst_rel (along partitions)
    """
    row = np.asarray(edge_index[0]).astype(np.int64)
    col = np.asarray(edge_index[1]).astype(np.int64)
    if remap is not None:
        row = remap(row)
    if half is None:
        half = cfg.HALF

    # per shard: dict[(tile, bucket)] -> (idx16 [n], drel [n])
    groups = []
    for q in range(8):
        base = q * cfg.SHARD
        m = (col >= base) & (col < base + cfg.SHARD)
        r = row[m]
        c = col[m] - base
        t = c // P
        drel = c % P
        bkt = (r >= half).astype(np.int64)
        order = np.lexsort((r, drel, bkt, t))
        r, t, drel, bkt = r[order], t[order], drel[order], bkt[order]
        idx16 = np.where(bkt == 1, r - half, r).astype(np.int16)
        g = {}
        key = t * 2 + bkt
        bounds = np.flatnonzero(np.append(True, key[1:] != key[:-1]))
        bounds = np.append(bounds, len(key))
        for j in range(len(bounds) - 1):
            s, e = int(bounds[j]), int(bounds[j + 1])
            g[(int(t[s]), int(bkt[s]))] = (idx16[s:e], drel[s:e].astype(np.float32))
        groups.append(g)

    # canonical chunk counts: per (tile, bucket) max over shards
    kmax = {}
    for t in range(cfg.TILES):
        for b in (0, 1):
            n = max(len(g.get((t, b), ((), ()))[0]) for g in groups)
            kmax[(t, b)] = max(1, math.ceil(n / P))

    calls = []
    tile_chunks = np.zeros(cfg.TILES, np.int64)
    chunk_of = {}  # (tile,bucket) -> first chunk index
    c0 = 0
    icol = 0
    for t in range(cfg.TILES):
        for b in (0, 1):
            k = kmax[(t, b)]
            chunk_of[(t, b)] = c0
            tile_chunks[t] += k
            for s2 in range(0, k, CPC):
                nch = min(CPC, k - s2)
                calls.append((t, b, c0 + s2, nch, icol))
                icol += nch * 8
            c0 += k
    nch_total = c0

    wi_list, wd_list = [], []
    for q in range(8):
        wi = np.zeros((128, nch_total * 8), np.int16)
        wd = np.full((128, nch_total), 255.0, np.float32)
        for (t, b), cc0 in chunk_of.items():
            idx16, drel = groups[q].get((t, b), (np.zeros(0, np.int16),
                                                 np.zeros(0, np.float32)))
            k = kmax[(t, b)]
            pi = np.zeros(k * P, np.int16)
            pd = np.full(k * P, 255.0, np.float32)
            pi[:len(idx16)] = idx16
            pd[:len(drel)] = drel
            wi[:, cc0 * 8:(cc0 + k) * 8] = _wrap16(pi)
            wd[:, cc0:cc0 + k] = pd.reshape(k, P).T
        wi_list.append(wi)
        wd_list.append(wd)
    return calls, tile_chunks, wi_list, wd_list


# ---------------------------------------------------------------- bass kernel


def build_nc(cfg, scheds, shapes):
    import concourse.bacc as bacc
    import concourse.bass as bass
    import concourse.tile as tile
    from concourse import mybir

    f32 = mybir.dt.float32
    bf16 = mybir.dt.bfloat16
    i16 = mybir.dt.int16
    tdt = bf16 if cfg.BF16 else f32
    TW = 2 * FEAT  # interleaved table width (256)

    nc = bacc.Bacc("TRN2", target_bir_lowering=False, debug=False,
                   num_devices=8, num_swdge_queues=4)

    x_tab = nc.dram_tensor("x_tab", [cfg.N, TW], tdt, kind="ExternalInput")
    wix_d = nc.dram_tensor("wix", list(shapes["wix"]), i16, kind="ExternalInput")
    wih_d = nc.dram_tensor("wih", list(shapes["wih"]), i16, kind="ExternalInput")
    wdx_d = nc.dram_tensor("wdx", list(shapes["wdx"]), f32, kind="ExternalInput")
    wdh_d = nc.dram_tensor("wdh", list(shapes["wdh"]), f32, kind="ExternalInput")
    w_dr = [nc.dram_tensor(f"W{i+1}", [FEAT, FEAT if i < 2 else 1], f32,
                           kind="ExternalInput") for i in range(3)]
    b_dr = [nc.dram_tensor(f"b{i+1}", [FEAT if i < 2 else 1], f32,
                           kind="ExternalInput") for i in range(3)]
    gb_dr = [(nc.dram_tensor(f"gamma{i+1}", [FEAT], f32, kind="ExternalInput"),
              nc.dram_tensor(f"beta{i+1}", [FEAT], f32, kind="ExternalInput"))
             for i in range(2)]
    iota_p_d = nc.dram_tensor("iota_p", [P, P], f32, kind="ExternalInput")
    ident_d = nc.dram_tensor("ident", [P, P], f32, kind="ExternalInput")
    out_d = nc.dram_tensor("out", [cfg.BATCH, cfg.SHARD], f32, kind="ExternalOutput")

    htab = [nc.dram_tensor(f"htab{i}", [cfg.N, TW], tdt, kind="Internal")
            for i in range(2)]
    shard_out = [nc.dram_tensor(f"shard_out{i}", [cfg.SHARD, TW], tdt,
                                kind="Internal") for i in range(2)]
    stat_in = [nc.dram_tensor(f"stat_in{i}", [P, 2], f32, kind="Internal")
               for i in range(2)]
    stat_out = [nc.dram_tensor(f"stat_out{i}", [P, 2], f32, kind="Internal")
                for i in range(2)]

    AluOp = mybir.AluOpType
    ActF = mybir.ActivationFunctionType

    def bcast_inner(ap, inner):
        return bass.AP(tensor=ap.tensor, offset=ap.offset,
                       ap=[list(ap.ap[0]), list(ap.ap[1]), [0, inner]])

    def bcast_rep(ap, reps):
        return bass.AP(tensor=ap.tensor, offset=ap.offset,
                       ap=[list(ap.ap[0]), [0, reps], list(ap.ap[1])])

    with tile.TileContext(nc) as tc:
        with (
            tc.tile_pool(name="consts", bufs=1) as consts,
            tc.tile_pool(name="gw", bufs=6) as gwp,
            tc.tile_pool(name="ohp", bufs=6) as ohp,
            tc.tile_pool(name="aggp", bufs=2) as aggp,
            tc.tile_pool(name="hraw", bufs=1) as hrawp,
            tc.tile_pool(name="aglo", bufs=1) as aglop,
            tc.tile_pool(name="statp", bufs=2) as statp,
            tc.tile_pool(name="small", bufs=8) as small,
            tc.tile_pool(name="p2", bufs=3) as p2p,
            tc.tile_pool(name="outp", bufs=1) as outp,
            tc.tile_pool(name="ps_agg", bufs=2, space="PSUM") as ps_agg,
            tc.tile_pool(name="ps_h", bufs=2, space="PSUM") as ps_h,
            tc.tile_pool(name="ps_t", bufs=2, space="PSUM") as ps_t,
        ):
            wix_sb = consts.tile(list(shapes["wix"]), i16, tag="wix")
            nc.sync.dma_start(out=wix_sb[:], in_=wix_d[:])
            wih_sb = consts.tile(list(shapes["wih"]), i16, tag="wih")
            nc.sync.dma_start(out=wih_sb[:], in_=wih_d[:])
            wdx_sb = consts.tile(list(shapes["wdx"]), f32, tag="wdx")
            nc.sync.dma_start(out=wdx_sb[:], in_=wdx_d[:])
            wdh_sb = consts.tile(list(shapes["wdh"]), f32, tag="wdh")
            nc.sync.dma_start(out=wdh_sb[:], in_=wdh_d[:])
            w_sb = []
            for i, wdr in enumerate(w_dr):
                t = consts.tile([P, FEAT if i < 2 else 1], f32, tag=f"w{i}")
                nc.sync.dma_start(out=t[:], in_=wdr[:])
                w_sb.append(t)
            b_sb = []
            for i, bd in enumerate(b_dr):
                t = consts.tile([P, 1], f32, tag=f"b{i}")
                if i < 2:
                    nc.sync.dma_start(out=t[:], in_=bd[:, None])
                else:
                    nc.sync.dma_start(out=t[:], in_=bd[:].to_broadcast([P, 1]))
                b_sb.append(t)
            gb_sb = []
            for i, (gd, bd) in enumerate(gb_dr):
                tg = consts.tile([P, 1], f32, tag=f"g{i}")
                nc.sync.dma_start(out=tg[:], in_=gd[:, None])
                tb = consts.tile([P, 1], f32, tag=f"be{i}")
                nc.sync.dma_start(out=tb[:], in_=bd[:, None])
                gb_sb.append((tg, tb))
            iota_p = consts.tile([P, P], f32, tag="iota_p")
            nc.sync.dma_start(out=iota_p[:], in_=iota_p_d[:])
            ident = consts.tile([P, P], f32, tag="ident")
            nc.sync.dma_start(out=ident[:], in_=ident_d[:])
            eps_sb = consts.tile([P, 1], f32, tag="eps")
            nc.vector.memset(eps_sb[:], cfg.EPS)

            for layer in range(cfg.LAYERS):
                table = x_tab if layer == 0 else htab[layer - 1]
                calls, tile_chunks, wi_sb, wd_sb = (
                    (scheds[0][0], scheds[0][1], wix_sb, wdx_sb) if layer == 0
                    else (scheds[1][0], scheds[1][1], wih_sb, wdh_sb))
                half = cfg.HALF if layer == 0 else 8 * 25 * P
                is_last = layer == cfg.LAYERS - 1
                if not is_last:
                    hraw = [hrawp.tile([P, cfg.TILES * P], f32,
                                       tag=f"hraw{b}", name=f"hraw{b}")
                            for b in range(2)]
                    stat_t = statp.tile([P, 2 * cfg.TILES, 6], f32, tag="stats")
                else:
                    out_sb = [outp.tile([P, cfg.TILES], f32,
                                        tag=f"outsb{b}", name=f"outsb{b}")
                              for b in range(2)]

                # chunks per (tile, bucket)
                kcnt = {}
                for (tt, bb, c0, nch, icol) in calls:
                    kcnt[(tt, bb)] = kcnt.get((tt, bb), 0) + nch
                agg_lo = [aglop.tile([P, cfg.TILES * P], f32,
                                     tag=f"aglo{b}", name=f"aglo{b}")
                          for b in range(2)]
                cur_tile = -1
                done_chunks = 0
                agg_ps = None
                qn = 0
                # phase A: all lo-bucket calls (dep: first AllGather half
                # only), flushed to the agg_lo slab; phase B: hi-bucket
                # calls, combined with the slab.
                phased = ([c for c in calls if c[1] == 0]
                          + [c for c in calls if c[1] == 1])
                for (tt, bb, c0, nch, icol) in phased:
                    if (tt, bb) != cur_tile:
                        cur_tile = (tt, bb)
                        done_chunks = 0
                        agg_ps = [ps_agg.tile([P, P], f32, tag=f"agg{b}",
                                              name=f"agg{b}")
                                  for b in range(2)]
                    gt = gwp.tile([P, CPC, TW], tdt, tag="gw")
                    src = table[0:half, :] if bb == 0 else table[half:, :]
                    nc.gpsimd.dma_gather(
                        gt[:, :nch, :], src,
                        wi_sb[:, icol:icol + nch * 8],
                        nch * P, nch * P, TW,
                        queue_num=qn,
                    )
                    qn = (qn + 1) % 4
                    oh = ohp.tile([P, CPC * P], tdt, tag="oh")
                    nc.vector.tensor_tensor(
                        out=oh[:, :nch * P],
                        in0=bcast_inner(wd_sb[:, c0:c0 + nch], P),
                        in1=bcast_rep(iota_p[:], nch),
                        op=AluOp.is_equal,
                    )
                    total = int(kcnt[(tt, bb)])
                    for j in range(nch):
                        first = done_chunks == 0
                        last = done_chunks == total - 1
                        for b in range(2):
                            nc.tensor.matmul(
                                agg_ps[b][:, :],
                                lhsT=gt[:, j, b * FEAT:(b + 1) * FEAT],
                                rhs=oh[:, j * P:(j + 1) * P],
                                start=first, stop=last,
                            )
                        done_chunks += 1

                    if done_chunks == total and bb == 0:
                        for b in range(2):
                            nc.vector.tensor_copy(
                                out=agg_lo[b][:, tt * P:(tt + 1) * P],
                                in_=agg_ps[b][:],
                            )
                    if done_chunks == total and bb == 1:
                        valid = cfg.VALID_LAST if tt == cfg.TILES - 1 else P
                        for b in range(2):
                            agg_sb = aggp.tile([P, P], f32, tag=f"aggsb{b}")
                            nc.vector.tensor_add(
                                out=agg_sb[:], in0=agg_ps[b][:],
                                in1=agg_lo[b][:, tt * P:(tt + 1) * P],
                            )
                            if not is_last:
                                h_ps = ps_h.tile([P, P], f32, tag="hps")
                                nc.tensor.matmul(
                                    h_ps[:], lhsT=w_sb[layer][:], rhs=agg_sb[:],
                                    start=True, stop=True,
                                )
                                nc.vector.tensor_scalar_add(
                                    out=hraw[b][:, tt * P:tt * P + P],
                                    in0=h_ps[:], scalar1=b_sb[layer][:],
                                )
                                nc.vector.bn_stats(
                                    out=stat_t[:, 2 * tt + b, :],
                                    in_=hraw[b][:, tt * P:tt * P + valid],
                                )
                            else:
                                o_ps = ps_h.tile([P, P], f32, tag="hps")
                                o_ps = o_ps[:, 0:1]
                                nc.tensor.matmul(
                                    o_ps[:], lhsT=agg_sb[:], rhs=w_sb[2][:],
                                    start=True, stop=True,
                                )
                                nc.vector.tensor_scalar_add(
                                    out=out_sb[b][:, tt:tt + 1], in0=o_ps[:],
                                    scalar1=b_sb[2][:],
                                )

                if not is_last:
                    mv = small.tile([P, 2], f32, tag="mv")
                    nc.vector.bn_aggr(out=mv[:], in_=stat_t[:, :, :])
                    sloc = small.tile([P, 2], f32, tag="sloc")
                    nc.vector.tensor_copy(out=sloc[:, 0:1], in_=mv[:, 0:1])
                    nc.vector.tensor_tensor(
                        out=sloc[:, 1:2], in0=mv[:, 0:1], in1=mv[:, 0:1],
                        op=AluOp.mult,
                    )
                    nc.vector.tensor_add(
                        out=sloc[:, 1:2], in0=sloc[:, 1:2], in1=mv[:, 1:2]
                    )
                    nc.sync.dma_start(out=stat_in[layer][:], in_=sloc[:])
                    if cfg.USE_AR:
                        nc.gpsimd.collective_compute(
                            "AllReduce", AluOp.add,
                            replica_groups=[[0, 1, 2, 3, 4, 5, 6, 7]],
                            ins=[stat_in[layer][:]], outs=[stat_out[layer][:]],
                        )
                    else:
                        nc.sync.dma_start(out=stat_out[layer][:],
                                          in_=stat_in[layer][:])
                    sglob = small.tile([P, 2], f32, tag="sglob")
                    nc.sync.dma_start(out=sglob[:], in_=stat_out[layer][:])
                    nc.scalar.mul(out=sglob[:], in_=sglob[:],
                                  mul=0.125 if cfg.USE_AR else 1.0)
                    var = small.tile([P, 1], f32, tag="var")
                    nc.vector.tensor_tensor(
                        out=var[:], in0=sglob[:, 0:1], in1=sglob[:, 0:1],
                        op=AluOp.mult,
                    )
                    nc.vector.tensor_sub(out=var[:], in0=sglob[:, 1:2], in1=var[:])
                    rstd = small.tile([P, 1], f32, tag="rstd")
                    nc.scalar.activation(out=rstd[:], in_=var[:], func=ActF.Sqrt,
                                         bias=eps_sb[:])
                    nc.vector.reciprocal(out=rstd[:], in_=rstd[:])
                    scal = small.tile([P, 1], f32, tag="scal")
                    nc.vector.tensor_tensor(
                        out=scal[:], in0=gb_sb[layer][0][:], in1=rstd[:],
                        op=AluOp.mult,
                    )
                    shif = small.tile([P, 1], f32, tag="shif")
                    nc.vector.tensor_tensor(
                        out=shif[:], in0=sglob[:, 0:1], in1=scal[:], op=AluOp.mult,
                    )
                    nc.vector.tensor_sub(out=shif[:], in0=gb_sb[layer][1][:],
                                         in1=shif[:])
                    HT = 25  # first-half tiles (rows 0:3200)
                    R1 = HT * P
                    R2 = cfg.SHARD - R1
                    for half, (t0, t1) in enumerate(((0, HT), (HT, cfg.TILES))):
                        for t in range(t0, t1):
                            valid = cfg.VALID_LAST if t == cfg.TILES - 1 else P
                            for b in range(2):
                                hbn = p2p.tile([P, P], f32, tag="hbn")
                                nc.scalar.activation(
                                    out=hbn[:], in_=hraw[b][:, t * P:(t + 1) * P],
                                    func=ActF.Relu, bias=shif[:], scale=scal[:],
                                )
                                t_ps = ps_t.tile([P, P], f32, tag="tps")
                                nc.tensor.transpose(out=t_ps[:], in_=hbn[:],
                                                    identity=ident[:])
                                hrow = p2p.tile([P, P], tdt, tag="hrow")
                                nc.vector.tensor_copy(out=hrow[:], in_=t_ps[:])
                                nc.sync.dma_start(
                                    out=shard_out[layer][t * P:t * P + valid,
                                                         b * FEAT:(b + 1) * FEAT],
                                    in_=hrow[:valid, :],
                                )
                        if cfg.USE_AG:
                            ht = htab[layer]
                            if half == 0:
                                in_ap = shard_out[layer][0:R1, :]
                                out_ap = ht[0:8 * R1, :]
                            else:
                                in_ap = shard_out[layer][R1:cfg.SHARD, :]
                                out_ap = ht[8 * R1:cfg.N, :]
                            nc.gpsimd.collective_compute(
                                "AllGather", AluOp.bypass,
                                replica_groups=[[0, 1, 2, 3, 4, 5, 6, 7]],
                                ins=[in_ap], outs=[out_ap],
                            )
                    if not cfg.USE_AG:
                        nc.sync.dma_start(out=htab[layer][0:cfg.SHARD, :],
                                          in_=shard_out[layer][:])
                else:
                    nfull = cfg.TILES - 1
                    for b in range(2):
                        nc.sync.dma_start(
                            out=out_d[b, 0:nfull * P].rearrange(
                                "(t p) -> p t", p=P),
                            in_=out_sb[b][:, 0:nfull],
                        )
                        nc.sync.dma_start(
                            out=out_d[b, nfull * P:cfg.SHARD, None],
                            in_=out_sb[b][:cfg.VALID_LAST, nfull:nfull + 1],
                        )

    nc.compile()
    return nc


# ---------------------------------------------------------------- host + run


def run_gcn(cfg, inputs, trace=False):
    import ml_dtypes
    from concourse.bass_utils import run_bass_kernel_spmd

    tnp = ml_dtypes.bfloat16 if cfg.BF16 else np.float32
    x = np.asarray(inputs["x"], dtype=np.float32)
    edge_index = np.asarray(inputs["edge_index"])
    R1 = 25 * P  # 3200: first-half rows per shard in the htab block layout

    def remap_h(r):
        q, rr = r // cfg.SHARD, r % cfg.SHARD
        return np.where(rr < R1, q * R1 + rr,
                        8 * R1 + q * (cfg.SHARD - R1) + (rr - R1))

    calls_x, tcx, wix_list, wdx_list = build_schedule(cfg, edge_index)
    calls_h, tch, wih_list, wdh_list = build_schedule(cfg, edge_index, remap_h,
                                                      half=8 * R1)
    # dst-side chunk layout must match between the two schedules for wd to be
    # shared; bucket membership differs, so use the h-schedule's wd and ALSO
    # the x-schedule's own wd: keep both by merging into one wd of max width.
    shapes = {"wix": wix_list[0].shape, "wih": wih_list[0].shape,
              "wdx": wdx_list[0].shape, "wdh": wdh_list[0].shape}
    nc = build_nc(cfg, (
        (calls_x, tcx), (calls_h, tch)), shapes)

    # interleaved x table: row n = [x(b0,n,:) | x(b1,n,:)]
    x_il = np.concatenate([x[0], x[1]], axis=1).astype(tnp)
    iota_p = np.tile(np.arange(P, dtype=np.float32), (P, 1))
    ident = np.eye(P, dtype=np.float32)
    common = {
        "W1": np.asarray(inputs["W1"], np.float32),
        "W2": np.asarray(inputs["W2"], np.float32),
        "W3": np.asarray(inputs["W3"], np.float32),
        "b1": np.asarray(inputs["b1"], np.float32),
        "b2": np.asarray(inputs["b2"], np.float32),
        "b3": np.asarray(inputs["b3"], np.float32),
        "gamma1": np.asarray(inputs["gamma1"], np.float32),
        "beta1": np.asarray(inputs["beta1"], np.float32),
        "gamma2": np.asarray(inputs["gamma2"], np.float32),
        "beta2": np.asarray(inputs["beta2"], np.float32),
        "iota_p": iota_p,
        "ident": ident,
        "x_tab": x_il,
    }
    in_maps = []
    for c in range(8):
        m = dict(common)
        m["wix"] = wix_list[c]
        m["wih"] = wih_list[c]
        m["wdx"] = wdx_list[c]
        m["wdh"] = wdh_list[c]
        in_maps.append(m)

    try:
        res = run_bass_kernel_spmd(nc, in_maps, core_ids=list(range(8)), trace=trace)
    except ModuleNotFoundError:
        res = run_bass_kernel_spmd(nc, in_maps, core_ids=list(range(8)), trace=False)
    out = np.empty((cfg.BATCH, cfg.N), np.float32)
    for c in range(8):
        out[:, c * cfg.SHARD:(c + 1) * cfg.SHARD] = res.results[c]["out"]
    return out, res


def kernel(**inputs) -> np.ndarray:
    cfg = Cfg()
    out, _ = run_gcn(cfg, inputs, trace=False)
    return out


# revision 13
# speedup vs baseline: 1.4814x; 1.0302x over previous
"""ClusterGCN (3-layer GCN, sum-aggregation) on 8 Trainium2 NeuronCores.

Strategy (hardcoded for B=2, N=50000, F=H=128, E=800000, 8 cores):
  - core c: destination shard c (6250 nodes), BOTH batches. Tables are
    batch-interleaved [N, 256]: row n = [h(b0,n,:) | h(b1,n,:)] in bf16, so
    one dma_gather index fetches both batches' source rows (512B). The SWDGE
    gather cost is per-index (measured flat in element width up to 1KB), so
    interleaving halves system-wide descriptor work vs per-batch sharding.
  - Reassociate each layer: A @ (h @ W) == (A @ h) @ W: aggregate first
    (segment-sum over edges), then one dense 128x128 matmul per batch.
  - Edges sorted by (dst_tile, src_bucket, dst, src) into 128-slot chunks;
    each chunk -> one is_equal one-hot [slot, dst_rel] and two accumulating
    matmuls (one per batch) into PSUM agg tiles [feat, dst]. Sources split
    into lo/hi buckets (int16 gather indices, offset table views).
  - Gather calls are <=1024 idx (Q7 scratch limit), cycled over SWDGE
    queues 0..3 (different Q7 CPU pairs; ~25% faster than one queue).
  - SPMD: one instruction stream for all 8 cores -> the call schedule is
    canonical (per-(tile,bucket) max chunk count over shards); each shard
    pads its own chunks with idx 0 / dst_rel 255 (one-hot all-zero).
  - BatchNorm is training-mode over all B*N rows: per-core bn_stats/bn_aggr,
    then an 8-core AllReduce of (mean, E[x^2]).
  - After BN+ReLU tiles are transposed back row-major (bf16) and AllGathered
    (all 8 cores) into the next layer's gather table.
"""

import math

import numpy as np

P = 128
FEAT = 128
CPC = 8  # max chunks per gather call (8*128 = 1024 idx, Q7 scratch limit)


class Cfg:
    def __init__(self, n_nodes=50000, batch=2, eps=1e-5):
        self.N = n_nodes
        self.SHARD = n_nodes // 8  # 6250
        self.BATCH = batch
        self.HALF = n_nodes // 2
        assert self.HALF <= 32767
        self.TILES = math.ceil(self.SHARD / P)  # 49
        self.VALID_LAST = self.SHARD - (self.TILES - 1) * P  # 106
        self.EPS = eps
        self.LAYERS = 3
        self.USE_AR = True
        self.USE_AG = True
        self.BF16 = True


def _wrap16(stream):
    """[n] idx stream -> [128, n/16] wrapped col-major, replicated x8."""
    return np.tile(stream.reshape(-1, 16).T, (8, 1))


def build_schedule(cfg, edge_index, remap=None, half=None):
    """Canonical dst-sorted chunk schedule shared by all 8 shards.

    remap: optional vectorized fn mapping global source ids to table rows
    (used for the half-block htab layout of layers 2-3).

    Returns (calls, tile_chunks, wi_list, wd_list):
      calls: list of (tile, bucket, chunk0, nch, icol) gather calls
      tile_chunks: [TILES] chunks per tile
      wi_list[q]: [128, n_chunks*8] i16 wrapped gather idx for shard q
      wd_list[q]: [128, n_chunks] f32 per-chunk dst_rel (along partitions)
    """
    row = np.asarray(edge_index[0]).astype(np.int64)
    col = np.asarray(edge_index[1]).astype(np.int64)
    if remap is not None:
        row = remap(row)
    if half is None:
        half = cfg.HALF

    # per shard: dict[(tile, bucket)] -> (idx16 [n], drel [n])
    groups = []
    for q in range(8):
        base = q * cfg.SHARD
        m = (col >= base) & (col < base + cfg.SHARD)
        r = row[m]
        c = col[m] - base
        t = c // P
        drel = c % P
        bkt = (r >= half).astype(np.int64)
        order = np.lexsort((r, drel, bkt, t))
        r, t, drel, bkt = r[order], t[order], drel[order], bkt[order]
        idx16 = np.where(bkt == 1, r - half, r).astype(np.int16)
        g = {}
        key = t * 2 + bkt
        bounds = np.flatnonzero(np.append(True, key[1:] != key[:-1]))
        bounds = np.append(bounds, len(key))
        for j in range(len(bounds) - 1):
            s, e = int(bounds[j]), int(bounds[j + 1])
            g[(int(t[s]), int(bkt[s]))] = (idx16[s:e], drel[s:e].astype(np.float32))
        groups.append(g)

    # canonical chunk counts: per (tile, bucket) max over shards
    kmax = {}
    for t in range(cfg.TILES):
        for b in (0, 1):
            n = max(len(g.get((t, b), ((), ()))[0]) for g in groups)
            kmax[(t, b)] = max(1, math.ceil(n / P))

    calls = []
    tile_chunks = np.zeros(cfg.TILES, np.int64)
    chunk_of = {}  # (tile,bucket) -> first chunk index
    c0 = 0
    icol = 0
    for t in range(cfg.TILES):
        for b in (0, 1):
            k = kmax[(t, b)]
            chunk_of[(t, b)] = c0
            tile_chunks[t] += k
            for s2 in range(0, k, CPC):
                nch = min(CPC, k - s2)
                calls.append((t, b, c0 + s2, nch, icol))
                icol += nch * 8
            c0 += k
    nch_total = c0

    wi_list, wd_list = [], []
    for q in range(8):
        wi = np.zeros((128, nch_total * 8), np.int16)
        wd = np.full((128, nch_total), 255.0, np.float32)
        for (t, b), cc0 in chunk_of.items():
            idx16, drel = groups[q].get((t, b), (np.zeros(0, np.int16),
                                                 np.zeros(0, np.float32)))
            k = kmax[(t, b)]
            pi = np.zeros(k * P, np.int16)
            pd = np.full(k * P, 255.0, np.float32)
            pi[:len(idx16)] = idx16
            pd[:len(drel)] = drel
            wi[:, cc0 * 8:(cc0 + k) * 8] = _wrap16(pi)
            wd[:, cc0:cc0 + k] = pd.reshape(k, P).T
        wi_list.append(wi)
        wd_list.append(wd)
    return calls, tile_chunks, wi_list, wd_list


# ---------------------------------------------------------------- bass kernel


def build_nc(cfg, scheds, shapes):
    import concourse.bacc as bacc
    import concourse.bass as bass
    import concourse.tile as tile
    from concourse import mybir

    f32 = mybir.dt.float32
    bf16 = mybir.dt.bfloat16
    i16 = mybir.dt.int16
    tdt = bf16 if cfg.BF16 else f32
    TW = 2 * FEAT  # interleaved table width (256)

    nc = bacc.Bacc("TRN2", target_bir_lowering=False, debug=False,
                   num_devices=8, num_swdge_queues=4)

    x_tab = nc.dram_tensor("x_tab", [cfg.N, TW], tdt, kind="ExternalInput")
    wix_d = nc.dram_tensor("wix", list(shapes["wix"]), i16, kind="ExternalInput")
    wih_d = nc.dram_tensor("wih", list(shapes["wih"]), i16, kind="ExternalInput")
    wdx_d = nc.dram_tensor("wdx", list(shapes["wdx"]), f32, kind="ExternalInput")
    wdh_d = nc.dram_tensor("wdh", list(shapes["wdh"]), f32, kind="ExternalInput")
    w_dr = [nc.dram_tensor(f"W{i+1}", [FEAT, FEAT if i < 2 else 1], f32,
                           kind="ExternalInput") for i in range(3)]
    b_dr = [nc.dram_tensor(f"b{i+1}", [FEAT if i < 2 else 1], f32,
                           kind="ExternalInput") for i in range(3)]
    gb_dr = [(nc.dram_tensor(f"gamma{i+1}", [FEAT], f32, kind="ExternalInput"),
              nc.dram_tensor(f"beta{i+1}", [FEAT], f32, kind="ExternalInput"))
             for i in range(2)]
    iota_p_d = nc.dram_tensor("iota_p", [P, P], f32, kind="ExternalInput")
    ident_d = nc.dram_tensor("ident", [P, P], f32, kind="ExternalInput")
    out_d = nc.dram_tensor("out", [cfg.BATCH, cfg.SHARD], f32, kind="ExternalOutput")

    htab = [nc.dram_tensor(f"htab{i}", [cfg.N, TW], tdt, kind="Internal")
            for i in range(2)]
    shard_out = [nc.dram_tensor(f"shard_out{i}", [cfg.SHARD, TW], tdt,
                                kind="Internal") for i in range(2)]
    stat_in = [nc.dram_tensor(f"stat_in{i}", [P, 2], f32, kind="Internal")
               for i in range(2)]
    stat_out = [nc.dram_tensor(f"stat_out{i}", [P, 2], f32, kind="Internal")
                for i in range(2)]

    AluOp = mybir.AluOpType
    ActF = mybir.ActivationFunctionType

    def bcast_inner(ap, inner):
        return bass.AP(tensor=ap.tensor, offset=ap.offset,
                       ap=[list(ap.ap[0]), list(ap.ap[1]), [0, inner]])

    def bcast_rep(ap, reps):
        return bass.AP(tensor=ap.tensor, offset=ap.offset,
                       ap=[list(ap.ap[0]), [0, reps], list(ap.ap[1])])

    with tile.TileContext(nc) as tc:
        with (
            tc.tile_pool(name="consts", bufs=1) as consts,
            tc.tile_pool(name="gw", bufs=8) as gwp,
            tc.tile_pool(name="ohp", bufs=8) as ohp,
            tc.tile_pool(name="aggp", bufs=2) as aggp,
            tc.tile_pool(name="hraw", bufs=1) as hrawp,
            tc.tile_pool(name="aglo", bufs=1) as aglop,
            tc.tile_pool(name="statp", bufs=2) as statp,
            tc.tile_pool(name="small", bufs=8) as small,
            tc.tile_pool(name="p2", bufs=3) as p2p,
            tc.tile_pool(name="outp", bufs=1) as outp,
            tc.tile_pool(name="ps_agg", bufs=2, space="PSUM") as ps_agg,
            tc.tile_pool(name="ps_h", bufs=2, space="PSUM") as ps_h,
            tc.tile_pool(name="ps_t", bufs=2, space="PSUM") as ps_t,
        ):
            wix_sb = consts.tile(list(shapes["wix"]), i16, tag="wix")
            nc.sync.dma_start(out=wix_sb[:], in_=wix_d[:])
            wih_sb = consts.tile(list(shapes["wih"]), i16, tag="wih")
            nc.sync.dma_start(out=wih_sb[:], in_=wih_d[:])
            wdx_sb = consts.tile(list(shapes["wdx"]), f32, tag="wdx")
            nc.sync.dma_start(out=wdx_sb[:], in_=wdx_d[:])
            wdh_sb = consts.tile(list(shapes["wdh"]), f32, tag="wdh")
            nc.sync.dma_start(out=wdh_sb[:], in_=wdh_d[:])
            w_sb = []
            for i, wdr in enumerate(w_dr):
                t = consts.tile([P, FEAT if i < 2 else 1], f32, tag=f"w{i}")
                nc.sync.dma_start(out=t[:], in_=wdr[:])
                w_sb.append(t)
            b_sb = []
            for i, bd in enumerate(b_dr):
                t = consts.tile([P, 1], f32, tag=f"b{i}")
                if i < 2:
                    nc.sync.dma_start(out=t[:], in_=bd[:, None])
                else:
                    nc.sync.dma_start(out=t[:], in_=bd[:].to_broadcast([P, 1]))
                b_sb.append(t)
            gb_sb = []
            for i, (gd, bd) in enumerate(gb_dr):
                tg = consts.tile([P, 1], f32, tag=f"g{i}")
                nc.sync.dma_start(out=tg[:], in_=gd[:, None])
                tb = consts.tile([P, 1], f32, tag=f"be{i}")
                nc.sync.dma_start(out=tb[:], in_=bd[:, None])
                gb_sb.append((tg, tb))
            iota_p = consts.tile([P, P], f32, tag="iota_p")
            nc.sync.dma_start(out=iota_p[:], in_=iota_p_d[:])
            ident = consts.tile([P, P], f32, tag="ident")
            nc.sync.dma_start(out=ident[:], in_=ident_d[:])
            eps_sb = consts.tile([P, 1], f32, tag="eps")
            nc.vector.memset(eps_sb[:], cfg.EPS)

            for layer in range(cfg.LAYERS):
                table = x_tab if layer == 0 else htab[layer - 1]
                calls, tile_chunks, wi_sb, wd_sb = (
                    (scheds[0][0], scheds[0][1], wix_sb, wdx_sb) if layer == 0
                    else (scheds[1][0], scheds[1][1], wih_sb, wdh_sb))
                half = cfg.HALF if layer == 0 else 8 * 25 * P
                is_last = layer == cfg.LAYERS - 1
                if not is_last:
                    hraw = [hrawp.tile([P, cfg.TILES * P], f32,
                                       tag=f"hraw{b}", name=f"hraw{b}")
                            for b in range(2)]
                    stat_t = statp.tile([P, 2 * cfg.TILES, 6], f32, tag="stats")
                else:
                    out_sb = [outp.tile([P, cfg.TILES], f32,
                                        tag=f"outsb{b}", name=f"outsb{b}")
                              for b in range(2)]

                # chunks per (tile, bucket)
                kcnt = {}
                for (tt, bb, c0, nch, icol) in calls:
                    kcnt[(tt, bb)] = kcnt.get((tt, bb), 0) + nch
                agg_lo = [aglop.tile([P, cfg.TILES * P], f32,
                                     tag=f"aglo{b}", name=f"aglo{b}")
                          for b in range(2)]
                cur_tile = -1
                done_chunks = 0
                agg_ps = None
                qn = 0
                # phase A: all lo-bucket calls (dep: first AllGather half
                # only), flushed to the agg_lo slab; phase B: hi-bucket
                # calls, combined with the slab.
                phased = ([c for c in calls if c[1] == 0]
                          + [c for c in calls if c[1] == 1])
                for (tt, bb, c0, nch, icol) in phased:
                    if (tt, bb) != cur_tile:
                        cur_tile = (tt, bb)
                        done_chunks = 0
                        agg_ps = [ps_agg.tile([P, P], f32, tag=f"agg{b}",
                                              name=f"agg{b}")
                                  for b in range(2)]
                    gt = gwp.tile([P, CPC, TW], tdt, tag="gw")
                    src = table[0:half, :] if bb == 0 else table[half:, :]
                    nc.gpsimd.dma_gather(
                        gt[:, :nch, :], src,
                        wi_sb[:, icol:icol + nch * 8],
                        nch * P, nch * P, TW,
                        queue_num=qn,
                    )
                    qn = (qn + 1) % 4
                    oh = ohp.tile([P, CPC * P], tdt, tag="oh")
                    nc.vector.tensor_tensor(
                        out=oh[:, :nch * P],
                        in0=bcast_inner(wd_sb[:, c0:c0 + nch], P),
                        in1=bcast_rep(iota_p[:], nch),
                        op=AluOp.is_equal,
                    )
                    total = int(kcnt[(tt, bb)])
                    for j in range(nch):
                        first = done_chunks == 0
                        last = done_chunks == total - 1
                        for b in range(2):
                            nc.tensor.matmul(
                                agg_ps[b][:, :],
                                lhsT=gt[:, j, b * FEAT:(b + 1) * FEAT],
                                rhs=oh[:, j * P:(j + 1) * P],
                                start=first, stop=last,
                            )
                        done_chunks += 1

                    if done_chunks == total and bb == 0:
                        for b in range(2):
                            nc.vector.tensor_copy(
                                out=agg_lo[b][:, tt * P:(tt + 1) * P],
                                in_=agg_ps[b][:],
                            )
                    if done_chunks == total and bb == 1:
                        valid = cfg.VALID_LAST if tt == cfg.TILES - 1 else P
                        for b in range(2):
                            agg_sb = aggp.tile([P, P], f32, tag=f"aggsb{b}")
                            nc.vector.tensor_add(
                                out=agg_sb[:], in0=agg_ps[b][:],
                                in1=agg_lo[b][:, tt * P:(tt + 1) * P],
                            )
                            if not is_last:
                                h_ps = ps_h.tile([P, P], f32, tag="hps")
                                nc.tensor.matmul(
                                    h_ps[:], lhsT=w_sb[layer][:], rhs=agg_sb[:],
                                    start=True, stop=True,
                                )
                                nc.vector.tensor_scalar_add(
                                    out=hraw[b][:, tt * P:tt * P + P],
                                    in0=h_ps[:], scalar1=b_sb[layer][:],
                                )
                                nc.vector.bn_stats(
                                    out=stat_t[:, 2 * tt + b, :],
                                    in_=hraw[b][:, tt * P:tt * P + valid],
                                )
                            else:
                                o_ps = ps_h.tile([P, P], f32, tag="hps")
                                o_ps = o_ps[:, 0:1]
                                nc.tensor.matmul(
                                    o_ps[:], lhsT=agg_sb[:], rhs=w_sb[2][:],
                                    start=True, stop=True,
                                )
                                nc.vector.tensor_scalar_add(
                                    out=out_sb[b][:, tt:tt + 1], in0=o_ps[:],
                                    scalar1=b_sb[2][:],
                                )

                if not is_last:
                    mv = small.tile([P, 2], f32, tag="mv")
                    nc.vector.bn_aggr(out=mv[:], in_=stat_t[:, :, :])
                    sloc = small.tile([P, 2], f32, tag="sloc")
                    nc.vector.tensor_copy(out=sloc[:, 0:1], in_=mv[:, 0:1])
                    nc.vector.tensor_tensor(
                        out=sloc[:, 1:2], in0=mv[:, 0:1], in1=mv[:, 0:1],
                        op=AluOp.mult,
                    )
                    nc.vector.tensor_add(
                        out=sloc[:, 1:2], in0=sloc[:, 1:2], in1=mv[:, 1:2]
                    )
                    nc.sync.dma_start(out=stat_in[layer][:], in_=sloc[:])
                    if cfg.USE_AR:
                        nc.gpsimd.collective_compute(
                            "AllReduce", AluOp.add,
                            replica_groups=[[0, 1, 2, 3, 4, 5, 6, 7]],
                            ins=[stat_in[layer][:]], outs=[stat_out[layer][:]],
                        )
                    else:
                        nc.sync.dma_start(out=stat_out[layer][:],
                                          in_=stat_in[layer][:])
                    sglob = small.tile([P, 2], f32, tag="sglob")
                    nc.sync.dma_start(out=sglob[:], in_=stat_out[layer][:])
                    nc.scalar.mul(out=sglob[:], in_=sglob[:],
                                  mul=0.125 if cfg.USE_AR else 1.0)
                    var = small.tile([P, 1], f32, tag="var")
                    nc.vector.tensor_tensor(
                        out=var[:], in0=sglob[:, 0:1], in1=sglob[:, 0:1],
                        op=AluOp.mult,
                    )
                    nc.vector.tensor_sub(out=var[:], in0=sglob[:, 1:2], in1=var[:])
                    rstd = small.tile([P, 1], f32, tag="rstd")
                    nc.scalar.activation(out=rstd[:], in_=var[:], func=ActF.Sqrt,
                                         bias=eps_sb[:])
                    nc.vector.reciprocal(out=rstd[:], in_=rstd[:])
                    scal = small.tile([P, 1], f32, tag="scal")
                    nc.vector.tensor_tensor(
                        out=scal[:], in0=gb_sb[layer][0][:], in1=rstd[:],
                        op=AluOp.mult,
                    )
                    shif = small.tile([P, 1], f32, tag="shif")
                    nc.vector.tensor_tensor(
                        out=shif[:], in0=sglob[:, 0:1], in1=scal[:], op=AluOp.mult,
                    )
                    nc.vector.tensor_sub(out=shif[:], in0=gb_sb[layer][1][:],
                                         in1=shif[:])
                    HT = 25  # first-half tiles (rows 0:3200)
                    R1 = HT * P
                    R2 = cfg.SHARD - R1
                    for half, (t0, t1) in enumerate(((0, HT), (HT, cfg.TILES))):
                        for t in range(t0, t1):
                            valid = cfg.VALID_LAST if t == cfg.TILES - 1 else P
                            for b in range(2):
                                hbn = p2p.tile([P, P], f32, tag="hbn")
                                nc.scalar.activation(
                                    out=hbn[:], in_=hraw[b][:, t * P:(t + 1) * P],
                                    func=ActF.Relu, bias=shif[:], scale=scal[:],
                                )
                                t_ps = ps_t.tile([P, P], f32, tag="tps")
                                nc.tensor.transpose(out=t_ps[:], in_=hbn[:],
                                                    identity=ident[:])
                                hrow = p2p.tile([P, P], tdt, tag="hrow")
                                nc.vector.tensor_copy(out=hrow[:], in_=t_ps[:])
                                nc.sync.dma_start(
                                    out=shard_out[layer][t * P:t * P + valid,
                                                         b * FEAT:(b + 1) * FEAT],
                                    in_=hrow[:valid, :],
                                )
                        if cfg.USE_AG:
                            ht = htab[layer]
                            if half == 0:
                                in_ap = shard_out[layer][0:R1, :]
                                out_ap = ht[0:8 * R1, :]
                            else:
                                in_ap = shard_out[layer][R1:cfg.SHARD, :]
                                out_ap = ht[8 * R1:cfg.N, :]
                            nc.gpsimd.collective_compute(
                                "AllGather", AluOp.bypass,
                                replica_groups=[[0, 1, 2, 3, 4, 5, 6, 7]],
                                ins=[in_ap], outs=[out_ap],
                            )
                    if not cfg.USE_AG:
                        nc.sync.dma_start(out=htab[layer][0:cfg.SHARD, :],
                                          in_=shard_out[layer][:])
                else:
                    nfull = cfg.TILES - 1
                    for b in range(2):
                        nc.sync.dma_start(
                            out=out_d[b, 0:nfull * P].rearrange(
                                "(t p) -> p t", p=P),
                            in_=out_sb[b][:, 0:nfull],
                        )
                        nc.sync.dma_start(
                            out=out_d[b, nfull * P:cfg.SHARD, None],
                            in_=out_sb[b][:cfg.VALID_LAST, nfull:nfull + 1],
                        )

    nc.compile()
    return nc


# ---------------------------------------------------------------- host + run


def run_gcn(cfg, inputs, trace=False):
    import ml_dtypes
    from concourse.bass_utils import run_bass_kernel_spmd

    tnp = ml_dtypes.bfloat16 if cfg.BF16 else np.float32
    x = np.asarray(inputs["x"], dtype=np.float32)
    edge_index = np.asarray(inputs["edge_index"])
    R1 = 25 * P  # 3200: first-half rows per shard in the htab block layout

    def remap_h(r):
        q, rr = r // cfg.SHARD, r % cfg.SHARD
        return np.where(rr < R1, q * R1 + rr,
                        8 * R1 + q * (cfg.SHARD - R1) + (rr - R1))

    calls_x, tcx, wix_list, wdx_list = build_schedule(cfg, edge_index)
    calls_h, tch, wih_list, wdh_list = build_schedule(cfg, edge_index, remap_h,
                                                      half=8 * R1)
    # dst-side chunk layout must match between the two schedules for wd to be
    # shared; bucket membership differs, so use the h-schedule's wd and ALSO
    # the x-schedule's own wd: keep both by merging into one wd of max width.
    shapes = {"wix": wix_list[0].shape, "wih": wih_list[0].shape,
              "wdx": wdx_list[0].shape, "wdh": wdh_list[0].shape}
    nc = build_nc(cfg, (
        (calls_x, tcx), (calls_h, tch)), shapes)

    # interleaved x table: row n = [x(b0,n,:) | x(b1,n,:)]
    x_il = np.concatenate([x[0], x[1]], axis=1).astype(tnp)
    iota_p = np.tile(np.arange(P, dtype=np.float32), (P, 1))
    ident = np.eye(P, dtype=np.float32)
    common = {
        "W1": np.asarray(inputs["W1"], np.float32),
        "W2": np.asarray(inputs["W2"], np.float32),
        "W3": np.asarray(inputs["W3"], np.float32),
        "b1": np.asarray(inputs["b1"], np.float32),
        "b2": np.asarray(inputs["b2"], np.float32),
        "b3": np.asarray(inputs["b3"], np.float32),
        "gamma1": np.asarray(inputs["gamma1"], np.float32),
        "beta1": np.asarray(inputs["beta1"], np.float32),
        "gamma2": np.asarray(inputs["gamma2"], np.float32),
        "beta2": np.asarray(inputs["beta2"], np.float32),
        "iota_p": iota_p,
        "ident": ident,
        "x_tab": x_il,
    }
    in_maps = []
    for c in range(8):
        m = dict(common)
        m["wix"] = wix_list[c]
        m["wih"] = wih_list[c]
        m["wdx"] = wdx_list[c]
        m["wdh"] = wdh_list[c]
        in_maps.append(m)

    try:
        res = run_bass_kernel_spmd(nc, in_maps, core_ids=list(range(8)), trace=trace)
    except ModuleNotFoundError:
        res = run_bass_kernel_spmd(nc, in_maps, core_ids=list(range(8)), trace=False)
    out = np.empty((cfg.BATCH, cfg.N), np.float32)
    for c in range(8):
        out[:, c * cfg.SHARD:(c + 1) * cfg.SHARD] = res.results[c]["out"]
    return out, res


def kernel(**inputs) -> np.ndarray:
    cfg = Cfg()
    out, _ = run_gcn(cfg, inputs, trace=False)
    return out


# revision 14
# speedup vs baseline: 1.4954x; 1.0094x over previous
"""ClusterGCN (3-layer GCN, sum-aggregation) on 8 Trainium2 NeuronCores.

Strategy (hardcoded for B=2, N=50000, F=H=128, E=800000, 8 cores):
  - core c: destination shard c (6250 nodes), BOTH batches. Tables are
    batch-interleaved [N, 256]: row n = [h(b0,n,:) | h(b1,n,:)] in bf16, so
    one dma_gather index fetches both batches' source rows (512B). The SWDGE
    gather cost is per-index (measured flat in element width up to 1KB), so
    interleaving halves system-wide descriptor work vs per-batch sharding.
  - Reassociate each layer: A @ (h @ W) == (A @ h) @ W: aggregate first
    (segment-sum over edges), then one dense 128x128 matmul per batch.
  - Edges sorted by (dst_tile, src_bucket, dst, src) into 128-slot chunks;
    each chunk -> one is_equal one-hot [slot, dst_rel] and two accumulating
    matmuls (one per batch) into PSUM agg tiles [feat, dst]. Sources split
    into lo/hi buckets (int16 gather indices, offset table views).
  - Gather calls are <=1024 idx (Q7 scratch limit), cycled over SWDGE
    queues 0..3 (different Q7 CPU pairs; ~25% faster than one queue).
  - SPMD: one instruction stream for all 8 cores -> the call schedule is
    canonical (per-(tile,bucket) max chunk count over shards); each shard
    pads its own chunks with idx 0 / dst_rel 255 (one-hot all-zero).
  - BatchNorm is training-mode over all B*N rows: per-core bn_stats/bn_aggr,
    then an 8-core AllReduce of (mean, E[x^2]).
  - After BN+ReLU tiles are transposed back row-major (bf16) and AllGathered
    (all 8 cores) into the next layer's gather table.
"""

import math

import numpy as np

P = 128
FEAT = 128
CPC = 8  # max chunks per gather call (8*128 = 1024 idx, Q7 scratch limit)


class Cfg:
    def __init__(self, n_nodes=50000, batch=2, eps=1e-5):
        self.N = n_nodes
        self.SHARD = n_nodes // 8  # 6250
        self.BATCH = batch
        self.HALF = n_nodes // 2
        assert self.HALF <= 32767
        self.TILES = math.ceil(self.SHARD / P)  # 49
        self.VALID_LAST = self.SHARD - (self.TILES - 1) * P  # 106
        self.EPS = eps
        self.LAYERS = 3
        self.USE_AR = True
        self.USE_AG = True
        self.BF16 = True


def _wrap16(stream):
    """[n] idx stream -> [128, n/16] wrapped col-major, replicated x8."""
    return np.tile(stream.reshape(-1, 16).T, (8, 1))


def build_schedule(cfg, edge_index, remap=None, half=None):
    """Canonical dst-sorted chunk schedule shared by all 8 shards.

    remap: optional vectorized fn mapping global source ids to table rows
    (used for the half-block htab layout of layers 2-3).

    Returns (calls, tile_chunks, wi_list, wd_list):
      calls: list of (tile, bucket, chunk0, nch, icol) gather calls
      tile_chunks: [TILES] chunks per tile
      wi_list[q]: [128, n_chunks*8] i16 wrapped gather idx for shard q
      wd_list[q]: [128, n_chunks] f32 per-chunk dst_rel (along partitions)
    """
    row = np.asarray(edge_index[0]).astype(np.int64)
    col = np.asarray(edge_index[1]).astype(np.int64)
    if remap is not None:
        row = remap(row)
    if half is None:
        half = cfg.HALF

    # per shard: dict[(tile, bucket)] -> (idx16 [n], drel [n])
    groups = []
    for q in range(8):
        base = q * cfg.SHARD
        m = (col >= base) & (col < base + cfg.SHARD)
        r = row[m]
        c = col[m] - base
        t = c // P
        drel = c % P
        bkt = (r >= half).astype(np.int64)
        order = np.lexsort((r, drel, bkt, t))
        r, t, drel, bkt = r[order], t[order], drel[order], bkt[order]
        idx16 = np.where(bkt == 1, r - half, r).astype(np.int16)
        g = {}
        key = t * 2 + bkt
        bounds = np.flatnonzero(np.append(True, key[1:] != key[:-1]))
        bounds = np.append(bounds, len(key))
        for j in range(len(bounds) - 1):
            s, e = int(bounds[j]), int(bounds[j + 1])
            g[(int(t[s]), int(bkt[s]))] = (idx16[s:e], drel[s:e].astype(np.float32))
        groups.append(g)

    # canonical chunk counts: per (tile, bucket) max over shards
    kmax = {}
    for t in range(cfg.TILES):
        for b in (0, 1):
            n = max(len(g.get((t, b), ((), ()))[0]) for g in groups)
            kmax[(t, b)] = max(1, math.ceil(n / P))

    calls = []
    tile_chunks = np.zeros(cfg.TILES, np.int64)
    chunk_of = {}  # (tile,bucket) -> first chunk index
    c0 = 0
    icol = 0
    for t in range(cfg.TILES):
        for b in (0, 1):
            k = kmax[(t, b)]
            chunk_of[(t, b)] = c0
            tile_chunks[t] += k
            for s2 in range(0, k, CPC):
                nch = min(CPC, k - s2)
                calls.append((t, b, c0 + s2, nch, icol))
                icol += nch * 8
            c0 += k
    nch_total = c0

    wi_list, wd_list = [], []
    for q in range(8):
        wi = np.zeros((128, nch_total * 8), np.int16)
        wd = np.full((128, nch_total), 255.0, np.float32)
        for (t, b), cc0 in chunk_of.items():
            idx16, drel = groups[q].get((t, b), (np.zeros(0, np.int16),
                                                 np.zeros(0, np.float32)))
            k = kmax[(t, b)]
            pi = np.zeros(k * P, np.int16)
            pd = np.full(k * P, 255.0, np.float32)
            pi[:len(idx16)] = idx16
            pd[:len(drel)] = drel
            wi[:, cc0 * 8:(cc0 + k) * 8] = _wrap16(pi)
            wd[:, cc0:cc0 + k] = pd.reshape(k, P).T
        wi_list.append(wi)
        wd_list.append(wd)
    return calls, tile_chunks, wi_list, wd_list


# ---------------------------------------------------------------- bass kernel


def build_nc(cfg, scheds, shapes):
    import concourse.bacc as bacc
    import concourse.bass as bass
    import concourse.tile as tile
    from concourse import mybir

    f32 = mybir.dt.float32
    bf16 = mybir.dt.bfloat16
    i16 = mybir.dt.int16
    tdt = bf16 if cfg.BF16 else f32
    TW = 2 * FEAT  # interleaved table width (256)

    nc = bacc.Bacc("TRN2", target_bir_lowering=False, debug=False,
                   num_devices=8, num_swdge_queues=4)

    x_tab = nc.dram_tensor("x_tab", [cfg.N, TW], tdt, kind="ExternalInput")
    wix_d = nc.dram_tensor("wix", list(shapes["wix"]), i16, kind="ExternalInput")
    wih_d = nc.dram_tensor("wih", list(shapes["wih"]), i16, kind="ExternalInput")
    wdx_d = nc.dram_tensor("wdx", list(shapes["wdx"]), f32, kind="ExternalInput")
    wdh_d = nc.dram_tensor("wdh", list(shapes["wdh"]), f32, kind="ExternalInput")
    w_dr = [nc.dram_tensor(f"W{i+1}", [FEAT, FEAT if i < 2 else 1], f32,
                           kind="ExternalInput") for i in range(3)]
    b_dr = [nc.dram_tensor(f"b{i+1}", [FEAT if i < 2 else 1], f32,
                           kind="ExternalInput") for i in range(3)]
    gb_dr = [(nc.dram_tensor(f"gamma{i+1}", [FEAT], f32, kind="ExternalInput"),
              nc.dram_tensor(f"beta{i+1}", [FEAT], f32, kind="ExternalInput"))
             for i in range(2)]
    iota_p_d = nc.dram_tensor("iota_p", [P, P], f32, kind="ExternalInput")
    ident_d = nc.dram_tensor("ident", [P, P], f32, kind="ExternalInput")
    out_d = nc.dram_tensor("out", [cfg.BATCH, cfg.SHARD], f32, kind="ExternalOutput")

    htab = [nc.dram_tensor(f"htab{i}", [cfg.N, TW], tdt, kind="Internal")
            for i in range(2)]
    shard_out = [nc.dram_tensor(f"shard_out{i}", [cfg.SHARD, TW], tdt,
                                kind="Internal") for i in range(2)]
    stat_in = [nc.dram_tensor(f"stat_in{i}", [P, 2], f32, kind="Internal")
               for i in range(2)]
    stat_out = [nc.dram_tensor(f"stat_out{i}", [P, 2], f32, kind="Internal")
                for i in range(2)]

    AluOp = mybir.AluOpType
    ActF = mybir.ActivationFunctionType

    def bcast_inner(ap, inner):
        return bass.AP(tensor=ap.tensor, offset=ap.offset,
                       ap=[list(ap.ap[0]), list(ap.ap[1]), [0, inner]])

    def bcast_rep(ap, reps):
        return bass.AP(tensor=ap.tensor, offset=ap.offset,
                       ap=[list(ap.ap[0]), [0, reps], list(ap.ap[1])])

    with tile.TileContext(nc) as tc:
        with (
            tc.tile_pool(name="consts", bufs=1) as consts,
            tc.tile_pool(name="gw", bufs=8) as gwp,
            tc.tile_pool(name="ohp", bufs=8) as ohp,
            tc.tile_pool(name="aggp", bufs=4) as aggp,
            tc.tile_pool(name="hraw", bufs=1) as hrawp,
            tc.tile_pool(name="aglo", bufs=1) as aglop,
            tc.tile_pool(name="statp", bufs=2) as statp,
            tc.tile_pool(name="small", bufs=8) as small,
            tc.tile_pool(name="p2", bufs=6) as p2p,
            tc.tile_pool(name="outp", bufs=1) as outp,
            tc.tile_pool(name="ps_agg", bufs=2, space="PSUM") as ps_agg,
            tc.tile_pool(name="ps_h", bufs=2, space="PSUM") as ps_h,
            tc.tile_pool(name="ps_t", bufs=2, space="PSUM") as ps_t,
        ):
            wix_sb = consts.tile(list(shapes["wix"]), i16, tag="wix")
            nc.sync.dma_start(out=wix_sb[:], in_=wix_d[:])
            wih_sb = consts.tile(list(shapes["wih"]), i16, tag="wih")
            nc.sync.dma_start(out=wih_sb[:], in_=wih_d[:])
            wdx_sb = consts.tile(list(shapes["wdx"]), f32, tag="wdx")
            nc.sync.dma_start(out=wdx_sb[:], in_=wdx_d[:])
            wdh_sb = consts.tile(list(shapes["wdh"]), f32, tag="wdh")
            nc.sync.dma_start(out=wdh_sb[:], in_=wdh_d[:])
            w_sb = []
            for i, wdr in enumerate(w_dr):
                t = consts.tile([P, FEAT if i < 2 else 1], f32, tag=f"w{i}")
                nc.sync.dma_start(out=t[:], in_=wdr[:])
                w_sb.append(t)
            b_sb = []
            for i, bd in enumerate(b_dr):
                t = consts.tile([P, 1], f32, tag=f"b{i}")
                if i < 2:
                    nc.sync.dma_start(out=t[:], in_=bd[:, None])
                else:
                    nc.sync.dma_start(out=t[:], in_=bd[:].to_broadcast([P, 1]))
                b_sb.append(t)
            gb_sb = []
            for i, (gd, bd) in enumerate(gb_dr):
                tg = consts.tile([P, 1], f32, tag=f"g{i}")
                nc.sync.dma_start(out=tg[:], in_=gd[:, None])
                tb = consts.tile([P, 1], f32, tag=f"be{i}")
                nc.sync.dma_start(out=tb[:], in_=bd[:, None])
                gb_sb.append((tg, tb))
            iota_p = consts.tile([P, P], f32, tag="iota_p")
            nc.sync.dma_start(out=iota_p[:], in_=iota_p_d[:])
            ident = consts.tile([P, P], f32, tag="ident")
            nc.sync.dma_start(out=ident[:], in_=ident_d[:])
            eps_sb = consts.tile([P, 1], f32, tag="eps")
            nc.vector.memset(eps_sb[:], cfg.EPS)

            for layer in range(cfg.LAYERS):
                table = x_tab if layer == 0 else htab[layer - 1]
                calls, tile_chunks, wi_sb, wd_sb = (
                    (scheds[0][0], scheds[0][1], wix_sb, wdx_sb) if layer == 0
                    else (scheds[1][0], scheds[1][1], wih_sb, wdh_sb))
                half = cfg.HALF if layer == 0 else 8 * 25 * P
                is_last = layer == cfg.LAYERS - 1
                if not is_last:
                    hraw = [hrawp.tile([P, cfg.TILES * P], f32,
                                       tag=f"hraw{b}", name=f"hraw{b}")
                            for b in range(2)]
                    stat_t = statp.tile([P, 2 * cfg.TILES, 6], f32, tag="stats")
                else:
                    out_sb = [outp.tile([P, cfg.TILES], f32,
                                        tag=f"outsb{b}", name=f"outsb{b}")
                              for b in range(2)]

                # chunks per (tile, bucket)
                kcnt = {}
                for (tt, bb, c0, nch, icol) in calls:
                    kcnt[(tt, bb)] = kcnt.get((tt, bb), 0) + nch
                agg_lo = [aglop.tile([P, cfg.TILES * P], f32,
                                     tag=f"aglo{b}", name=f"aglo{b}")
                          for b in range(2)]
                cur_tile = -1
                done_chunks = 0
                agg_ps = None
                qn = 0
                # phase A: all lo-bucket calls (dep: first AllGather half
                # only), flushed to the agg_lo slab; phase B: hi-bucket
                # calls, combined with the slab.
                phased = ([c for c in calls if c[1] == 0]
                          + [c for c in calls if c[1] == 1])
                for (tt, bb, c0, nch, icol) in phased:
                    if (tt, bb) != cur_tile:
                        cur_tile = (tt, bb)
                        done_chunks = 0
                        agg_ps = [ps_agg.tile([P, P], f32, tag=f"agg{b}",
                                              name=f"agg{b}")
                                  for b in range(2)]
                    gt = gwp.tile([P, CPC, TW], tdt, tag="gw")
                    src = table[0:half, :] if bb == 0 else table[half:, :]
                    nc.gpsimd.dma_gather(
                        gt[:, :nch, :], src,
                        wi_sb[:, icol:icol + nch * 8],
                        nch * P, nch * P, TW,
                        queue_num=qn,
                    )
                    qn = (qn + 1) % 4
                    oh = ohp.tile([P, CPC * P], tdt, tag="oh")
                    nc.vector.tensor_tensor(
                        out=oh[:, :nch * P],
                        in0=bcast_inner(wd_sb[:, c0:c0 + nch], P),
                        in1=bcast_rep(iota_p[:], nch),
                        op=AluOp.is_equal,
                    )
                    total = int(kcnt[(tt, bb)])
                    for j in range(nch):
                        first = done_chunks == 0
                        last = done_chunks == total - 1
                        for b in range(2):
                            nc.tensor.matmul(
                                agg_ps[b][:, :],
                                lhsT=gt[:, j, b * FEAT:(b + 1) * FEAT],
                                rhs=oh[:, j * P:(j + 1) * P],
                                start=first, stop=last,
                            )
                        done_chunks += 1

                    if done_chunks == total and bb == 0:
                        for b in range(2):
                            nc.vector.tensor_copy(
                                out=agg_lo[b][:, tt * P:(tt + 1) * P],
                                in_=agg_ps[b][:],
                            )
                    if done_chunks == total and bb == 1:
                        valid = cfg.VALID_LAST if tt == cfg.TILES - 1 else P
                        for b in range(2):
                            agg_sb = aggp.tile([P, P], f32, tag=f"aggsb{b}")
                            nc.vector.tensor_add(
                                out=agg_sb[:], in0=agg_ps[b][:],
                                in1=agg_lo[b][:, tt * P:(tt + 1) * P],
                            )
                            if not is_last:
                                h_ps = ps_h.tile([P, P], f32, tag="hps")
                                nc.tensor.matmul(
                                    h_ps[:], lhsT=w_sb[layer][:], rhs=agg_sb[:],
                                    start=True, stop=True,
                                )
                                nc.vector.tensor_scalar_add(
                                    out=hraw[b][:, tt * P:tt * P + P],
                                    in0=h_ps[:], scalar1=b_sb[layer][:],
                                )
                                nc.vector.bn_stats(
                                    out=stat_t[:, 2 * tt + b, :],
                                    in_=hraw[b][:, tt * P:tt * P + valid],
                                )
                            else:
                                o_ps = ps_h.tile([P, P], f32, tag="hps")
                                o_ps = o_ps[:, 0:1]
                                nc.tensor.matmul(
                                    o_ps[:], lhsT=agg_sb[:], rhs=w_sb[2][:],
                                    start=True, stop=True,
                                )
                                nc.vector.tensor_scalar_add(
                                    out=out_sb[b][:, tt:tt + 1], in0=o_ps[:],
                                    scalar1=b_sb[2][:],
                                )

                if not is_last:
                    mv = small.tile([P, 2], f32, tag="mv")
                    nc.vector.bn_aggr(out=mv[:], in_=stat_t[:, :, :])
                    sloc = small.tile([P, 2], f32, tag="sloc")
                    nc.vector.tensor_copy(out=sloc[:, 0:1], in_=mv[:, 0:1])
                    nc.vector.tensor_tensor(
                        out=sloc[:, 1:2], in0=mv[:, 0:1], in1=mv[:, 0:1],
                        op=AluOp.mult,
                    )
                    nc.vector.tensor_add(
                        out=sloc[:, 1:2], in0=sloc[:, 1:2], in1=mv[:, 1:2]
                    )
                    nc.sync.dma_start(out=stat_in[layer][:], in_=sloc[:])
                    if cfg.USE_AR:
                        nc.gpsimd.collective_compute(
                            "AllReduce", AluOp.add,
                            replica_groups=[[0, 1, 2, 3, 4, 5, 6, 7]],
                            ins=[stat_in[layer][:]], outs=[stat_out[layer][:]],
                        )
                    else:
                        nc.sync.dma_start(out=stat_out[layer][:],
                                          in_=stat_in[layer][:])
                    sglob = small.tile([P, 2], f32, tag="sglob")
                    nc.sync.dma_start(out=sglob[:], in_=stat_out[layer][:])
                    nc.scalar.mul(out=sglob[:], in_=sglob[:],
                                  mul=0.125 if cfg.USE_AR else 1.0)
                    var = small.tile([P, 1], f32, tag="var")
                    nc.vector.tensor_tensor(
                        out=var[:], in0=sglob[:, 0:1], in1=sglob[:, 0:1],
                        op=AluOp.mult,
                    )
                    nc.vector.tensor_sub(out=var[:], in0=sglob[:, 1:2], in1=var[:])
                    rstd = small.tile([P, 1], f32, tag="rstd")
                    nc.scalar.activation(out=rstd[:], in_=var[:], func=ActF.Sqrt,
                                         bias=eps_sb[:])
                    nc.vector.reciprocal(out=rstd[:], in_=rstd[:])
                    scal = small.tile([P, 1], f32, tag="scal")
                    nc.vector.tensor_tensor(
                        out=scal[:], in0=gb_sb[layer][0][:], in1=rstd[:],
                        op=AluOp.mult,
                    )
                    shif = small.tile([P, 1], f32, tag="shif")
                    nc.vector.tensor_tensor(
                        out=shif[:], in0=sglob[:, 0:1], in1=scal[:], op=AluOp.mult,
                    )
                    nc.vector.tensor_sub(out=shif[:], in0=gb_sb[layer][1][:],
                                         in1=shif[:])
                    HT = 25  # first-half tiles (rows 0:3200)
                    R1 = HT * P
                    R2 = cfg.SHARD - R1
                    for half, (t0, t1) in enumerate(((0, HT), (HT, cfg.TILES))):
                        for t in range(t0, t1):
                            valid = cfg.VALID_LAST if t == cfg.TILES - 1 else P
                            for b in range(2):
                                hbn = p2p.tile([P, P], f32, tag="hbn")
                                nc.scalar.activation(
                                    out=hbn[:], in_=hraw[b][:, t * P:(t + 1) * P],
                                    func=ActF.Relu, bias=shif[:], scale=scal[:],
                                )
                                t_ps = ps_t.tile([P, P], f32, tag="tps")
                                nc.tensor.transpose(out=t_ps[:], in_=hbn[:],
                                                    identity=ident[:])
                                hrow = p2p.tile([P, P], tdt, tag="hrow")
                                nc.vector.tensor_copy(out=hrow[:], in_=t_ps[:])
                                nc.sync.dma_start(
                                    out=shard_out[layer][t * P:t * P + valid,
                                                         b * FEAT:(b + 1) * FEAT],
                                    in_=hrow[:valid, :],
                                )
                        if cfg.USE_AG:
                            ht = htab[layer]
                            if half == 0:
                                in_ap = shard_out[layer][0:R1, :]
                                out_ap = ht[0:8 * R1, :]
                            else:
                                in_ap = shard_out[layer][R1:cfg.SHARD, :]
                                out_ap = ht[8 * R1:cfg.N, :]
                            nc.gpsimd.collective_compute(
                                "AllGather", AluOp.bypass,
                                replica_groups=[[0, 1, 2, 3, 4, 5, 6, 7]],
                                ins=[in_ap], outs=[out_ap],
                            )
                    if not cfg.USE_AG:
                        nc.sync.dma_start(out=htab[layer][0:cfg.SHARD, :],
                                          in_=shard_out[layer][:])
                else:
                    nfull = cfg.TILES - 1
                    for b in range(2):
                        nc.sync.dma_start(
                            out=out_d[b, 0:nfull * P].rearrange(
                                "(t p) -> p t", p=P),
                            in_=out_sb[b][:, 0:nfull],
                        )
                        nc.sync.dma_start(
                            out=out_d[b, nfull * P:cfg.SHARD, None],
                            in_=out_sb[b][:cfg.VALID_LAST, nfull:nfull + 1],
                        )

    nc.compile()
    return nc


# ---------------------------------------------------------------- host + run


def run_gcn(cfg, inputs, trace=False):
    import ml_dtypes
    from concourse.bass_utils import run_bass_kernel_spmd

    tnp = ml_dtypes.bfloat16 if cfg.BF16 else np.float32
    x = np.asarray(inputs["x"], dtype=np.float32)
    edge_index = np.asarray(inputs["edge_index"])
    R1 = 25 * P  # 3200: first-half rows per shard in the htab block layout

    def remap_h(r):
        q, rr = r // cfg.SHARD, r % cfg.SHARD
        return np.where(rr < R1, q * R1 + rr,
                        8 * R1 + q * (cfg.SHARD - R1) + (rr - R1))

    calls_x, tcx, wix_list, wdx_list = build_schedule(cfg, edge_index)
    calls_h, tch, wih_list, wdh_list = build_schedule(cfg, edge_index, remap_h,
                                                      half=8 * R1)
    # dst-side chunk layout must match between the two schedules for wd to be
    # shared; bucket membership differs, so use the h-schedule's wd and ALSO
    # the x-schedule's own wd: keep both by merging into one wd of max width.
    shapes = {"wix": wix_list[0].shape, "wih": wih_list[0].shape,
              "wdx": wdx_list[0].shape, "wdh": wdh_list[0].shape}
    nc = build_nc(cfg, (
        (calls_x, tcx), (calls_h, tch)), shapes)

    # interleaved x table: row n = [x(b0,n,:) | x(b1,n,:)]
    x_il = np.concatenate([x[0], x[1]], axis=1).astype(tnp)
    iota_p = np.tile(np.arange(P, dtype=np.float32), (P, 1))
    ident = np.eye(P, dtype=np.float32)
    common = {
        "W1": np.asarray(inputs["W1"], np.float32),
        "W2": np.asarray(inputs["W2"], np.float32),
        "W3": np.asarray(inputs["W3"], np.float32),
        "b1": np.asarray(inputs["b1"], np.float32),
        "b2": np.asarray(inputs["b2"], np.float32),
        "b3": np.asarray(inputs["b3"], np.float32),
        "gamma1": np.asarray(inputs["gamma1"], np.float32),
        "beta1": np.asarray(inputs["beta1"], np.float32),
        "gamma2": np.asarray(inputs["gamma2"], np.float32),
        "beta2": np.asarray(inputs["beta2"], np.float32),
        "iota_p": iota_p,
        "ident": ident,
        "x_tab": x_il,
    }
    in_maps = []
    for c in range(8):
        m = dict(common)
        m["wix"] = wix_list[c]
        m["wih"] = wih_list[c]
        m["wdx"] = wdx_list[c]
        m["wdh"] = wdh_list[c]
        in_maps.append(m)

    try:
        res = run_bass_kernel_spmd(nc, in_maps, core_ids=list(range(8)), trace=trace)
    except ModuleNotFoundError:
        res = run_bass_kernel_spmd(nc, in_maps, core_ids=list(range(8)), trace=False)
    out = np.empty((cfg.BATCH, cfg.N), np.float32)
    for c in range(8):
        out[:, c * cfg.SHARD:(c + 1) * cfg.SHARD] = res.results[c]["out"]
    return out, res


def kernel(**inputs) -> np.ndarray:
    cfg = Cfg()
    out, _ = run_gcn(cfg, inputs, trace=False)
    return out


# revision 15
# speedup vs baseline: 1.5773x; 1.0548x over previous
"""ClusterGCN (3-layer GCN, sum-aggregation) on 8 Trainium2 NeuronCores.

Strategy (hardcoded for B=2, N=50000, F=H=128, E=800000, 8 cores):
  - core c: destination shard c (6250 nodes), BOTH batches. Tables are
    batch-interleaved [N, 256]: row n = [h(b0,n,:) | h(b1,n,:)] in bf16, so
    one dma_gather index fetches both batches' source rows (512B). The SWDGE
    gather cost is per-index (measured flat in element width up to 1KB), so
    interleaving halves system-wide descriptor work vs per-batch sharding.
  - Reassociate each layer: A @ (h @ W) == (A @ h) @ W: aggregate first
    (segment-sum over edges), then one dense 128x128 matmul per batch.
  - Edges sorted by (dst_tile, src_bucket, dst, src) into 128-slot chunks;
    each chunk -> one is_equal one-hot [slot, dst_rel] and two accumulating
    matmuls (one per batch) into PSUM agg tiles [feat, dst]. Sources split
    into lo/hi buckets (int16 gather indices, offset table views).
  - Gather calls are <=1024 idx (Q7 scratch limit), cycled over SWDGE
    queues 0..3 (different Q7 CPU pairs; ~25% faster than one queue).
  - SPMD: one instruction stream for all 8 cores -> the call schedule is
    canonical (per-(tile,bucket) max chunk count over shards); each shard
    pads its own chunks with idx 0 / dst_rel 255 (one-hot all-zero).
  - BatchNorm is training-mode over all B*N rows: per-core bn_stats/bn_aggr,
    then an 8-core AllReduce of (mean, E[x^2]).
  - After BN+ReLU tiles are transposed back row-major (bf16) and AllGathered
    (all 8 cores) into the next layer's gather table.
"""

import math

import numpy as np

P = 128
FEAT = 128
CPC = 8  # max chunks per gather call (8*128 = 1024 idx, Q7 scratch limit)


class Cfg:
    def __init__(self, n_nodes=50000, batch=2, eps=1e-5):
        self.N = n_nodes
        self.SHARD = n_nodes // 8  # 6250
        self.BATCH = batch
        self.HALF = n_nodes // 2
        assert self.HALF <= 32767
        self.TILES = math.ceil(self.SHARD / P)  # 49
        self.VALID_LAST = self.SHARD - (self.TILES - 1) * P  # 106
        self.EPS = eps
        self.LAYERS = 3
        self.USE_AR = True
        self.USE_AG = True
        self.BF16 = True


def _wrap16(stream):
    """[n] idx stream -> [128, n/16] wrapped col-major, replicated x8."""
    return np.tile(stream.reshape(-1, 16).T, (8, 1))


def build_schedule(cfg, edge_index, remap=None, half=None):
    """Canonical dst-sorted chunk schedule shared by all 8 shards.

    remap: optional vectorized fn mapping global source ids to table rows
    (used for the half-block htab layout of layers 2-3).

    Returns (calls, tile_chunks, wi_list, wd_list):
      calls: list of (tile, bucket, chunk0, nch, icol) gather calls
      tile_chunks: [TILES] chunks per tile
      wi_list[q]: [128, n_chunks*8] i16 wrapped gather idx for shard q
      wd_list[q]: [128, n_chunks] f32 per-chunk dst_rel (along partitions)
    """
    row = np.asarray(edge_index[0]).astype(np.int64)
    col = np.asarray(edge_index[1]).astype(np.int64)
    if remap is not None:
        row = remap(row)
    if half is None:
        half = cfg.HALF

    # per shard: dict[(tile, bucket)] -> (idx16 [n], drel [n])
    groups = []
    for q in range(8):
        base = q * cfg.SHARD
        m = (col >= base) & (col < base + cfg.SHARD)
        r = row[m]
        c = col[m] - base
        t = c // P
        drel = c % P
        bkt = (r >= half).astype(np.int64)
        order = np.lexsort((r, drel, bkt, t))
        r, t, drel, bkt = r[order], t[order], drel[order], bkt[order]
        idx16 = np.where(bkt == 1, r - half, r).astype(np.int16)
        g = {}
        key = t * 2 + bkt
        bounds = np.flatnonzero(np.append(True, key[1:] != key[:-1]))
        bounds = np.append(bounds, len(key))
        for j in range(len(bounds) - 1):
            s, e = int(bounds[j]), int(bounds[j + 1])
            g[(int(t[s]), int(bkt[s]))] = (idx16[s:e], drel[s:e].astype(np.float32))
        groups.append(g)

    # canonical chunk counts: per (tile, bucket) max over shards
    kmax = {}
    for t in range(cfg.TILES):
        for b in (0, 1):
            n = max(len(g.get((t, b), ((), ()))[0]) for g in groups)
            kmax[(t, b)] = max(1, math.ceil(n / P))

    # bucket-major chunk numbering; pooled calls of exactly CPC chunks
    # (a call is just a gather container - chunks may span tiles)
    chunk_of = {}  # (tile,bucket) -> first chunk index
    chunk_meta = []  # per chunk: (tile, first_in_group, last_in_group)
    c0 = 0
    for b in (0, 1):
        for t in range(cfg.TILES):
            k = kmax[(t, b)]
            chunk_of[(t, b)] = c0
            for j in range(k):
                chunk_meta.append((t, j == 0, j == k - 1))
            c0 += k
    nch_total = c0
    calls = []  # (bucket, chunk0, nch)
    nlo = sum(kmax[(t, 0)] for t in range(cfg.TILES))
    for b, (lo, hi) in ((0, (0, nlo)), (1, (nlo, nch_total))):
        for s2 in range(lo, hi, CPC):
            calls.append((b, s2, min(CPC, hi - s2)))
    tile_chunks = chunk_meta  # repurposed return slot

    wi_list, wd_list = [], []
    for q in range(8):
        wi = np.zeros((128, nch_total * 8), np.int16)
        wd = np.full((128, nch_total), 255.0, np.float32)
        for (t, b), cc0 in chunk_of.items():
            idx16, drel = groups[q].get((t, b), (np.zeros(0, np.int16),
                                                 np.zeros(0, np.float32)))
            k = kmax[(t, b)]
            pi = np.zeros(k * P, np.int16)
            pd = np.full(k * P, 255.0, np.float32)
            pi[:len(idx16)] = idx16
            pd[:len(drel)] = drel
            wi[:, cc0 * 8:(cc0 + k) * 8] = _wrap16(pi)
            wd[:, cc0:cc0 + k] = pd.reshape(k, P).T
        wi_list.append(wi)
        wd_list.append(wd)
    return calls, tile_chunks, wi_list, wd_list


# ---------------------------------------------------------------- bass kernel


def build_nc(cfg, scheds, shapes):
    import concourse.bacc as bacc
    import concourse.bass as bass
    import concourse.tile as tile
    from concourse import mybir

    f32 = mybir.dt.float32
    bf16 = mybir.dt.bfloat16
    i16 = mybir.dt.int16
    tdt = bf16 if cfg.BF16 else f32
    TW = 2 * FEAT  # interleaved table width (256)

    nc = bacc.Bacc("TRN2", target_bir_lowering=False, debug=False,
                   num_devices=8, num_swdge_queues=4)

    x_tab = nc.dram_tensor("x_tab", [cfg.N, TW], tdt, kind="ExternalInput")
    wix_d = nc.dram_tensor("wix", list(shapes["wix"]), i16, kind="ExternalInput")
    wih_d = nc.dram_tensor("wih", list(shapes["wih"]), i16, kind="ExternalInput")
    wdx_d = nc.dram_tensor("wdx", list(shapes["wdx"]), f32, kind="ExternalInput")
    wdh_d = nc.dram_tensor("wdh", list(shapes["wdh"]), f32, kind="ExternalInput")
    w_dr = [nc.dram_tensor(f"W{i+1}", [FEAT, FEAT if i < 2 else 1], f32,
                           kind="ExternalInput") for i in range(3)]
    b_dr = [nc.dram_tensor(f"b{i+1}", [FEAT if i < 2 else 1], f32,
                           kind="ExternalInput") for i in range(3)]
    gb_dr = [(nc.dram_tensor(f"gamma{i+1}", [FEAT], f32, kind="ExternalInput"),
              nc.dram_tensor(f"beta{i+1}", [FEAT], f32, kind="ExternalInput"))
             for i in range(2)]
    iota_p_d = nc.dram_tensor("iota_p", [P, P], f32, kind="ExternalInput")
    ident_d = nc.dram_tensor("ident", [P, P], f32, kind="ExternalInput")
    out_d = nc.dram_tensor("out", [cfg.BATCH, cfg.SHARD], f32, kind="ExternalOutput")

    htab = [nc.dram_tensor(f"htab{i}", [cfg.N, TW], tdt, kind="Internal")
            for i in range(2)]
    shard_out = [nc.dram_tensor(f"shard_out{i}", [cfg.SHARD, TW], tdt,
                                kind="Internal") for i in range(2)]
    stat_in = [nc.dram_tensor(f"stat_in{i}", [P, 2], f32, kind="Internal")
               for i in range(2)]
    stat_out = [nc.dram_tensor(f"stat_out{i}", [P, 2], f32, kind="Internal")
                for i in range(2)]

    AluOp = mybir.AluOpType
    ActF = mybir.ActivationFunctionType

    def bcast_inner(ap, inner):
        return bass.AP(tensor=ap.tensor, offset=ap.offset,
                       ap=[list(ap.ap[0]), list(ap.ap[1]), [0, inner]])

    def bcast_rep(ap, reps):
        return bass.AP(tensor=ap.tensor, offset=ap.offset,
                       ap=[list(ap.ap[0]), [0, reps], list(ap.ap[1])])

    with tile.TileContext(nc) as tc:
        with (
            tc.tile_pool(name="consts", bufs=1) as consts,
            tc.tile_pool(name="gw", bufs=8) as gwp,
            tc.tile_pool(name="ohp", bufs=8) as ohp,
            tc.tile_pool(name="aggp", bufs=4) as aggp,
            tc.tile_pool(name="hraw", bufs=1) as hrawp,
            tc.tile_pool(name="aglo", bufs=1) as aglop,
            tc.tile_pool(name="statp", bufs=2) as statp,
            tc.tile_pool(name="small", bufs=8) as small,
            tc.tile_pool(name="p2", bufs=6) as p2p,
            tc.tile_pool(name="outp", bufs=1) as outp,
            tc.tile_pool(name="ps_agg", bufs=2, space="PSUM") as ps_agg,
            tc.tile_pool(name="ps_h", bufs=2, space="PSUM") as ps_h,
            tc.tile_pool(name="ps_t", bufs=2, space="PSUM") as ps_t,
        ):
            wix_sb = consts.tile(list(shapes["wix"]), i16, tag="wix")
            nc.sync.dma_start(out=wix_sb[:], in_=wix_d[:])
            wih_sb = consts.tile(list(shapes["wih"]), i16, tag="wih")
            nc.sync.dma_start(out=wih_sb[:], in_=wih_d[:])
            wdx_sb = consts.tile(list(shapes["wdx"]), f32, tag="wdx")
            nc.sync.dma_start(out=wdx_sb[:], in_=wdx_d[:])
            wdh_sb = consts.tile(list(shapes["wdh"]), f32, tag="wdh")
            nc.sync.dma_start(out=wdh_sb[:], in_=wdh_d[:])
            w_sb = []
            for i, wdr in enumerate(w_dr):
                t = consts.tile([P, FEAT if i < 2 else 1], f32, tag=f"w{i}")
                nc.sync.dma_start(out=t[:], in_=wdr[:])
                w_sb.append(t)
            b_sb = []
            for i, bd in enumerate(b_dr):
                t = consts.tile([P, 1], f32, tag=f"b{i}")
                if i < 2:
                    nc.sync.dma_start(out=t[:], in_=bd[:, None])
                else:
                    nc.sync.dma_start(out=t[:], in_=bd[:].to_broadcast([P, 1]))
                b_sb.append(t)
            gb_sb = []
            for i, (gd, bd) in enumerate(gb_dr):
                tg = consts.tile([P, 1], f32, tag=f"g{i}")
                nc.sync.dma_start(out=tg[:], in_=gd[:, None])
                tb = consts.tile([P, 1], f32, tag=f"be{i}")
                nc.sync.dma_start(out=tb[:], in_=bd[:, None])
                gb_sb.append((tg, tb))
            iota_p = consts.tile([P, P], f32, tag="iota_p")
            nc.sync.dma_start(out=iota_p[:], in_=iota_p_d[:])
            ident = consts.tile([P, P], f32, tag="ident")
            nc.sync.dma_start(out=ident[:], in_=ident_d[:])
            eps_sb = consts.tile([P, 1], f32, tag="eps")
            nc.vector.memset(eps_sb[:], cfg.EPS)

            for layer in range(cfg.LAYERS):
                table = x_tab if layer == 0 else htab[layer - 1]
                calls, tile_chunks, wi_sb, wd_sb = (
                    (scheds[0][0], scheds[0][1], wix_sb, wdx_sb) if layer == 0
                    else (scheds[1][0], scheds[1][1], wih_sb, wdh_sb))
                half = cfg.HALF if layer == 0 else 8 * 25 * P
                is_last = layer == cfg.LAYERS - 1
                if not is_last:
                    hraw = [hrawp.tile([P, cfg.TILES * P], f32,
                                       tag=f"hraw{b}", name=f"hraw{b}")
                            for b in range(2)]
                    stat_t = statp.tile([P, 2 * cfg.TILES, 6], f32, tag="stats")
                else:
                    out_sb = [outp.tile([P, cfg.TILES], f32,
                                        tag=f"outsb{b}", name=f"outsb{b}")
                              for b in range(2)]

                chunk_meta = tile_chunks  # (tile, first, last) per chunk
                agg_lo = [aglop.tile([P, cfg.TILES * P], f32,
                                     tag=f"aglo{b}", name=f"aglo{b}")
                          for b in range(2)]
                agg_ps = None
                qn = 0
                # phase A = lo-bucket calls (dep: first AllGather half only),
                # flushed to agg_lo; phase B = hi-bucket calls + slab combine.
                for (bb, c0, nch) in calls:
                    gt = gwp.tile([P, CPC, TW], tdt, tag="gw")
                    src = table[0:half, :] if bb == 0 else table[half:, :]
                    nc.gpsimd.dma_gather(
                        gt[:, :nch, :], src,
                        wi_sb[:, c0 * 8:(c0 + nch) * 8],
                        nch * P, nch * P, TW,
                        queue_num=qn,
                    )
                    qn = (qn + 1) % 4
                    oh = ohp.tile([P, CPC * P], tdt, tag="oh")
                    nc.vector.tensor_tensor(
                        out=oh[:, :nch * P],
                        in0=bcast_inner(wd_sb[:, c0:c0 + nch], P),
                        in1=bcast_rep(iota_p[:], nch),
                        op=AluOp.is_equal,
                    )
                    for j in range(nch):
                        tt, first, last = chunk_meta[c0 + j]
                        if first:
                            agg_ps = [ps_agg.tile([P, P], f32, tag=f"agg{b}",
                                                  name=f"agg{b}")
                                      for b in range(2)]
                        for b in range(2):
                            nc.tensor.matmul(
                                agg_ps[b][:, :],
                                lhsT=gt[:, j, b * FEAT:(b + 1) * FEAT],
                                rhs=oh[:, j * P:(j + 1) * P],
                                start=first, stop=last,
                            )
                        if last and bb == 0:
                            for b in range(2):
                                nc.vector.tensor_copy(
                                    out=agg_lo[b][:, tt * P:(tt + 1) * P],
                                    in_=agg_ps[b][:],
                                )
                        elif last and bb == 1:
                            valid = cfg.VALID_LAST if tt == cfg.TILES - 1 else P
                            for b in range(2):
                                agg_sb = aggp.tile([P, P], f32, tag=f"aggsb{b}")
                                nc.vector.tensor_add(
                                    out=agg_sb[:], in0=agg_ps[b][:],
                                    in1=agg_lo[b][:, tt * P:(tt + 1) * P],
                                )
                                if not is_last:
                                    h_ps = ps_h.tile([P, P], f32, tag="hps")
                                    nc.tensor.matmul(
                                        h_ps[:], lhsT=w_sb[layer][:],
                                        rhs=agg_sb[:], start=True, stop=True,
                                    )
                                    nc.vector.tensor_scalar_add(
                                        out=hraw[b][:, tt * P:tt * P + P],
                                        in0=h_ps[:], scalar1=b_sb[layer][:],
                                    )
                                    nc.vector.bn_stats(
                                        out=stat_t[:, 2 * tt + b, :],
                                        in_=hraw[b][:, tt * P:tt * P + valid],
                                    )
                                else:
                                    o_ps = ps_h.tile([P, P], f32, tag="hps")
                                    o_ps = o_ps[:, 0:1]
                                    nc.tensor.matmul(
                                        o_ps[:], lhsT=agg_sb[:], rhs=w_sb[2][:],
                                        start=True, stop=True,
                                    )
                                    nc.vector.tensor_scalar_add(
                                        out=out_sb[b][:, tt:tt + 1],
                                        in0=o_ps[:], scalar1=b_sb[2][:],
                                    )

                if not is_last:
                    mv = small.tile([P, 2], f32, tag="mv")
                    nc.vector.bn_aggr(out=mv[:], in_=stat_t[:, :, :])
                    sloc = small.tile([P, 2], f32, tag="sloc")
                    nc.vector.tensor_copy(out=sloc[:, 0:1], in_=mv[:, 0:1])
                    nc.vector.tensor_tensor(
                        out=sloc[:, 1:2], in0=mv[:, 0:1], in1=mv[:, 0:1],
                        op=AluOp.mult,
                    )
                    nc.vector.tensor_add(
                        out=sloc[:, 1:2], in0=sloc[:, 1:2], in1=mv[:, 1:2]
                    )
                    nc.sync.dma_start(out=stat_in[layer][:], in_=sloc[:])
                    if cfg.USE_AR:
                        nc.gpsimd.collective_compute(
                            "AllReduce", AluOp.add,
                            replica_groups=[[0, 1, 2, 3, 4, 5, 6, 7]],
                            ins=[stat_in[layer][:]], outs=[stat_out[layer][:]],
                        )
                    else:
                        nc.sync.dma_start(out=stat_out[layer][:],
                                          in_=stat_in[layer][:])
                    sglob = small.tile([P, 2], f32, tag="sglob")
                    nc.sync.dma_start(out=sglob[:], in_=stat_out[layer][:])
                    nc.scalar.mul(out=sglob[:], in_=sglob[:],
                                  mul=0.125 if cfg.USE_AR else 1.0)
                    var = small.tile([P, 1], f32, tag="var")
                    nc.vector.tensor_tensor(
                        out=var[:], in0=sglob[:, 0:1], in1=sglob[:, 0:1],
                        op=AluOp.mult,
                    )
                    nc.vector.tensor_sub(out=var[:], in0=sglob[:, 1:2], in1=var[:])
                    rstd = small.tile([P, 1], f32, tag="rstd")
                    nc.scalar.activation(out=rstd[:], in_=var[:], func=ActF.Sqrt,
                                         bias=eps_sb[:])
                    nc.vector.reciprocal(out=rstd[:], in_=rstd[:])
                    scal = small.tile([P, 1], f32, tag="scal")
                    nc.vector.tensor_tensor(
                        out=scal[:], in0=gb_sb[layer][0][:], in1=rstd[:],
                        op=AluOp.mult,
                    )
                    shif = small.tile([P, 1], f32, tag="shif")
                    nc.vector.tensor_tensor(
                        out=shif[:], in0=sglob[:, 0:1], in1=scal[:], op=AluOp.mult,
                    )
                    nc.vector.tensor_sub(out=shif[:], in0=gb_sb[layer][1][:],
                                         in1=shif[:])
                    HT = 25  # first-half tiles (rows 0:3200)
                    R1 = HT * P
                    R2 = cfg.SHARD - R1
                    for half, (t0, t1) in enumerate(((0, HT), (HT, cfg.TILES))):
                        for t in range(t0, t1):
                            valid = cfg.VALID_LAST if t == cfg.TILES - 1 else P
                            for b in range(2):
                                hbn = p2p.tile([P, P], f32, tag="hbn")
                                nc.scalar.activation(
                                    out=hbn[:], in_=hraw[b][:, t * P:(t + 1) * P],
                                    func=ActF.Relu, bias=shif[:], scale=scal[:],
                                )
                                t_ps = ps_t.tile([P, P], f32, tag="tps")
                                nc.tensor.transpose(out=t_ps[:], in_=hbn[:],
                                                    identity=ident[:])
                                hrow = p2p.tile([P, P], tdt, tag="hrow")
                                nc.vector.tensor_copy(out=hrow[:], in_=t_ps[:])
                                nc.sync.dma_start(
                                    out=shard_out[layer][t * P:t * P + valid,
                                                         b * FEAT:(b + 1) * FEAT],
                                    in_=hrow[:valid, :],
                                )
                        if cfg.USE_AG:
                            ht = htab[layer]
                            if half == 0:
                                in_ap = shard_out[layer][0:R1, :]
                                out_ap = ht[0:8 * R1, :]
                            else:
                                in_ap = shard_out[layer][R1:cfg.SHARD, :]
                                out_ap = ht[8 * R1:cfg.N, :]
                            nc.gpsimd.collective_compute(
                                "AllGather", AluOp.bypass,
                                replica_groups=[[0, 1, 2, 3, 4, 5, 6, 7]],
                                ins=[in_ap], outs=[out_ap],
                            )
                    if not cfg.USE_AG:
                        nc.sync.dma_start(out=htab[layer][0:cfg.SHARD, :],
                                          in_=shard_out[layer][:])
                else:
                    nfull = cfg.TILES - 1
                    for b in range(2):
                        nc.sync.dma_start(
                            out=out_d[b, 0:nfull * P].rearrange(
                                "(t p) -> p t", p=P),
                            in_=out_sb[b][:, 0:nfull],
                        )
                        nc.sync.dma_start(
                            out=out_d[b, nfull * P:cfg.SHARD, None],
                            in_=out_sb[b][:cfg.VALID_LAST, nfull:nfull + 1],
                        )

    nc.compile()
    return nc


# ---------------------------------------------------------------- host + run


def run_gcn(cfg, inputs, trace=False):
    import ml_dtypes
    from concourse.bass_utils import run_bass_kernel_spmd

    tnp = ml_dtypes.bfloat16 if cfg.BF16 else np.float32
    x = np.asarray(inputs["x"], dtype=np.float32)
    edge_index = np.asarray(inputs["edge_index"])
    R1 = 25 * P  # 3200: first-half rows per shard in the htab block layout

    def remap_h(r):
        q, rr = r // cfg.SHARD, r % cfg.SHARD
        return np.where(rr < R1, q * R1 + rr,
                        8 * R1 + q * (cfg.SHARD - R1) + (rr - R1))

    calls_x, tcx, wix_list, wdx_list = build_schedule(cfg, edge_index)
    calls_h, tch, wih_list, wdh_list = build_schedule(cfg, edge_index, remap_h,
                                                      half=8 * R1)
    # dst-side chunk layout must match between the two schedules for wd to be
    # shared; bucket membership differs, so use the h-schedule's wd and ALSO
    # the x-schedule's own wd: keep both by merging into one wd of max width.
    shapes = {"wix": wix_list[0].shape, "wih": wih_list[0].shape,
              "wdx": wdx_list[0].shape, "wdh": wdh_list[0].shape}
    nc = build_nc(cfg, (
        (calls_x, tcx), (calls_h, tch)), shapes)

    # interleaved x table: row n = [x(b0,n,:) | x(b1,n,:)]
    x_il = np.concatenate([x[0], x[1]], axis=1).astype(tnp)
    iota_p = np.tile(np.arange(P, dtype=np.float32), (P, 1))
    ident = np.eye(P, dtype=np.float32)
    common = {
        "W1": np.asarray(inputs["W1"], np.float32),
        "W2": np.asarray(inputs["W2"], np.float32),
        "W3": np.asarray(inputs["W3"], np.float32),
        "b1": np.asarray(inputs["b1"], np.float32),
        "b2": np.asarray(inputs["b2"], np.float32),
        "b3": np.asarray(inputs["b3"], np.float32),
        "gamma1": np.asarray(inputs["gamma1"], np.float32),
        "beta1": np.asarray(inputs["beta1"], np.float32),
        "gamma2": np.asarray(inputs["gamma2"], np.float32),
        "beta2": np.asarray(inputs["beta2"], np.float32),
        "iota_p": iota_p,
        "ident": ident,
        "x_tab": x_il,
    }
    in_maps = []
    for c in range(8):
        m = dict(common)
        m["wix"] = wix_list[c]
        m["wih"] = wih_list[c]
        m["wdx"] = wdx_list[c]
        m["wdh"] = wdh_list[c]
        in_maps.append(m)

    try:
        res = run_bass_kernel_spmd(nc, in_maps, core_ids=list(range(8)), trace=trace)
    except ModuleNotFoundError:
        res = run_bass_kernel_spmd(nc, in_maps, core_ids=list(range(8)), trace=False)
    out = np.empty((cfg.BATCH, cfg.N), np.float32)
    for c in range(8):
        out[:, c * cfg.SHARD:(c + 1) * cfg.SHARD] = res.results[c]["out"]
    return out, res


def kernel(**inputs) -> np.ndarray:
    cfg = Cfg()
    out, _ = run_gcn(cfg, inputs, trace=False)
    return out
